# revision 1
# baseline (speedup 1.0000x reference)
# Multi-head attention (N=2, T=2048, E=1024, H=16, DH=64) on 8 TRN2 NeuronCores.
#
# Sharding: tensor-parallel over heads x data-parallel over batch.
#   core c in 0..7 -> batch n = c // 4, heads [4*(c%4) .. 4*(c%4)+3].
# Each core computes its 4 heads' Q/K/V projections, causal attention, and a
# partial output projection (its slice of Wo). Host sums the 4 partials per
# batch and adds the bias.
#
# Device layouts (per core):
#   qT/kT/vT : [E, T] bf16   (host pre-transposes inputs; E on partitions)
#   QT/KT    : [128, T]      head-pair-stacked q^T/k^T (rows 0-63 head 2p, 64-127 head 2p+1)
#   V        : [128, 16*65]  v tiles per head with an appended ones column
#                            (PV matmul then yields the softmax denominator for free)
#   S^T      : [Tk=128, Tq=512] per block -> exp -> P^T, which feeds the PV
#              matmul directly as rhs (no transposes anywhere on device).
# Softmax skips max-subtraction: energies are pre-scaled by 1/sqrt(DH) (folded
# into Wq on host) and are O(+-10), far from fp32 exp overflow.

import os
from contextlib import ExitStack

import ml_dtypes
import numpy as np

import concourse.bass as bass
import concourse.mybir as mybir
import concourse.tile as tile
from concourse import bacc
from concourse.bass_utils import run_bass_kernel_spmd

N, T, E, H, DH = 2, 2048, 1024, 16, 64
P = 128
KC = E // P          # 8 contraction chunks for projections
TB = T // P          # 16 token blocks of 128
T4 = T // 512        # 4 token blocks of 512
HPC = 4              # heads per core
NCORES = 8
BF = mybir.dt.bfloat16
F32 = mybir.dt.float32
EXP = mybir.ActivationFunctionType.Exp

_PROG_CACHE: dict = {}


def _emit(ctx: ExitStack, tc: "tile.TileContext", io: dict, variant: str):
    nc = tc.nc
    const = ctx.enter_context(tc.tile_pool(name="const", bufs=1))
    xin = ctx.enter_context(tc.tile_pool(name="xin", bufs=1))
    proj = ctx.enter_context(tc.tile_pool(name="proj", bufs=1))
    work = ctx.enter_context(tc.tile_pool(name="work", bufs=3))
    psmm = ctx.enter_context(tc.tile_pool(name="psmm", bufs=2, space="PSUM"))
    psacc = ctx.enter_context(tc.tile_pool(name="psacc", bufs=2, space="PSUM"))

    # ---- load weights & constants (emission order ~ priority order) ----
    # One DMA per logical load (3D access patterns) to minimize per-DMA
    # HWDGE fixed overhead.
    def load_w(srcname, p):
        # host pre-swizzles to [128, KC*128]; contiguous 2 KB-line DMA
        w_t = const.tile([P, KC * P], BF, tag=f"{srcname}{p}", name=f"{srcname}{p}")
        nc.sync.dma_start(w_t, io[srcname][p])
        return w_t

    wq_sb = [load_w("wq", p) for p in range(2)]

    # qT/kT/vT as single tiles [128, KC*T], loaded in 512-token stripes
    # (one 3D DMA per (tensor, t4)) so the first projection groups only
    # wait for ~1 MB of DMA.
    def alloc_xt(key):
        t = xin.tile([P, KC * T], BF, tag=key, name=key)
        return t

    def xt_ap(t, kc, lo, hi):  # [128, hi-lo] chunk kc token-slice
        return t[:, kc * T + lo: kc * T + hi]

    def load_xt_t4(t, key, t4):
        dst = t.rearrange("p (c t) -> p c t", c=KC)[:, :, t4 * 512:(t4 + 1) * 512]
        src = io[key].rearrange("(c p) t -> p c t", p=P)[:, :, t4 * 512:(t4 + 1) * 512]
        nc.sync.dma_start(dst, src)

    qT_sb = alloc_xt("qT")
    kT_sb = alloc_xt("kT")
    vT_sb = alloc_xt("vT")
    wv_sb = const.tile([P, KC * 256], BF, tag="wv", name="wv")
    # DMA emission in first-use order: per-stripe q/k/v stripes, then weights
    # for later phases.
    load_xt_t4(qT_sb, "qT", 0)
    wk_sb = [load_w("wk", p) for p in range(2)]
    load_xt_t4(kT_sb, "kT", 0)
    nc.sync.dma_start(wv_sb, io["wv"])
    load_xt_t4(vT_sb, "vT", 0)
    if variant == "causal":
        sc_sb = const.tile([P, 896], BF, tag="sc", name="sc")
        nc.sync.dma_start(sc_sb, io["sc"])
    for t4 in range(1, T4):
        load_xt_t4(qT_sb, "qT", t4)
        load_xt_t4(kT_sb, "kT", t4)
        load_xt_t4(vT_sb, "vT", t4)
    woT_sb = []
    for cc in range(2):
        w_t = const.tile([P, E], BF, tag=f"woT{cc}", name=f"woT{cc}")
        nc.sync.dma_start(w_t, io["woT"][cc * P:(cc + 1) * P, :])
        woT_sb.append(w_t)

    # ---- projections ----
    QT = [proj.tile([P, T], BF, tag=f"QT{p}", name=f"QT{p}") for p in range(2)]
    KT = [proj.tile([P, T], BF, tag=f"KT{p}", name=f"KT{p}") for p in range(2)]
    # V for all 4 heads: per tb block, 4 heads x (64 cols + ones col)
    V4 = proj.tile([P, TB * HPC * 65], BF, tag="V4", name="V4")

    def V_ap(h, jk):  # [128, 65] lhsT for the PV matmul of head h, k-block jk
        return V4[:, jk * (HPC * 65) + h * 65: jk * (HPC * 65) + h * 65 + 65]

    CT = [proj.tile([P, T], BF, tag=f"CT{p}", name=f"CT{p}") for p in range(2)]

    v3 = V4.rearrange("p (b h c) -> p b h c", b=TB, h=HPC)
    nc.vector.memset(v3[:, :, :, 64:65], 1.0)

    def emit_qk_proj(t4):
        for which in range(2):  # all Q groups before K groups (DMA arrival order)
            for p in range(2):
                dst, w_sb, x_sb = ((QT[p], wq_sb[p], qT_sb), (KT[p], wk_sb[p], kT_sb))[which]
                ps = psmm.tile([P, 512], F32, tag="mmp", name="ps_proj")
                for kc in range(KC):
                    nc.tensor.matmul(
                        ps,
                        w_sb[:, kc * P:(kc + 1) * P],
                        xt_ap(x_sb, kc, t4 * 512, (t4 + 1) * 512),
                        start=(kc == 0),
                        stop=(kc == KC - 1),
                    )
                nc.scalar.copy(dst[:, t4 * 512:(t4 + 1) * 512], ps)

    def emit_v_proj(t4):
        for tb in range(4 * t4, 4 * t4 + 4):
            pv = psmm.tile([P, 256], F32, tag="mmp", name="ps_v")
            for kc in range(KC):
                nc.tensor.matmul(
                    pv,
                    xt_ap(vT_sb, kc, tb * P, (tb + 1) * P),
                    wv_sb[:, kc * 256:(kc + 1) * 256],
                    start=(kc == 0),
                    stop=(kc == KC - 1),
                )
            # Strided copy per tb: psum [128, 4*64] -> V4 head blocks (stride 65).
            nc.vector.tensor_copy(v3[:, tb, :, 0:64], pv.rearrange("p (h c) -> p h c", h=HPC))

    def emit_attention(iq):
        # Software-pipelined: the PV matmuls for block jk are emitted after
        # the S matmuls for block jk+1, so the PE never stalls on exp(jk).
        # Causal column trimming: for diagonal-overlap k-blocks (jk=4*iq+r),
        # query columns < 128*r are fully masked -> skipped in exp/mul/PV.
        njk = 4 * iq + 4 if variant == "causal" else TB
        for p in range(2):
            po = [psacc.tile([65, 512], F32, tag=f"po{hh}", name=f"po{hh}", bufs=1)
                  for hh in range(2)]
            pend = None  # (pt, co, jk) awaiting its PV matmuls

            def flush(pend, po=po, p=p, njk=njk):
                pt, co, jk = pend
                for hh in range(2):
                    h = 2 * p + hh
                    nc.tensor.matmul(
                        po[hh][:, co:512],
                        V_ap(h, jk),
                        pt[:, hh * 512 + co:(hh + 1) * 512],
                        start=(jk == 0),
                        stop=(jk == njk - 1),
                    )

            for jk in range(njk):
                r = jk - 4 * iq
                co = P * r if (variant == "causal" and r >= 0) else 0
                ps2 = psmm.tile([P, 1024], F32, tag="mm", name="ps_s")
                for hh in range(2):
                    nc.tensor.matmul(
                        ps2[:, hh * 512 + co:(hh + 1) * 512],
                        KT[p][hh * 64:(hh + 1) * 64, jk * P:(jk + 1) * P],
                        QT[p][hh * 64:(hh + 1) * 64, iq * 512 + co:(iq + 1) * 512],
                        start=True,
                        stop=True,
                    )
                pt = work.tile([P, 1024], BF, tag="pt", name="pt", bufs=4)
                src = ps2.rearrange("p (h q) -> p h q", h=2)[:, :, co:512]
                dst = pt.rearrange("p (h q) -> p h q", h=2)[:, :, co:512]
                nc.scalar.activation(dst, src, EXP)
                if variant == "causal" and r >= 0:
                    for hh in range(2):
                        sl = pt[:, hh * 512 + co:(hh + 1) * 512]
                        nc.vector.tensor_mul(sl, sl, sc_sb[:, 384:384 + 512 - co])
                elif variant == "mask":
                    mk = work.tile([P, 512], BF, tag="mk", name="mk")
                    nc.sync.dma_start(mk, io["mT"][jk * P:(jk + 1) * P, iq * 512:(iq + 1) * 512])
                    for hh in range(2):
                        sl = pt[:, hh * 512:(hh + 1) * 512]
                        nc.vector.tensor_mul(sl, sl, mk)
                if pend is not None:
                    flush(pend)
                pend = (pt, co, jk)
            flush(pend)

            for hh in range(2):
                rec = work.tile([1, 512], F32, tag="rec", name="rec")
                nc.vector.reciprocal(rec, po[hh][64:65, :])
                # partition-broadcast 1/denominator on GpSimd (idle engine)
                bc = work.tile([64, 512], F32, tag="bc", name="bc")
                nc.gpsimd.partition_broadcast(bc, rec)
                nc.vector.tensor_mul(
                    CT[p][hh * 64:(hh + 1) * 64, iq * 512:(iq + 1) * 512],
                    po[hh][0:64, :],
                    bc,
                )

    def emit_wo(iq):
        # output projection for one finished 512-token stripe
        for tb in range(4 * iq, 4 * iq + 4):
            yt = work.tile([P, 1024], BF, tag="yt", name="yt")
            for es in range(2):
                py = psmm.tile([P, 512], F32, tag="mmp", name="py")
                for cc in range(2):
                    nc.tensor.matmul(
                        py,
                        CT[cc][:, tb * P:(tb + 1) * P],
                        woT_sb[cc][:, es * 512:(es + 1) * 512],
                        start=(cc == 0),
                        stop=(cc == 1),
                    )
                if es == 0:
                    nc.scalar.copy(yt[:, 0:512], py)
                else:
                    nc.vector.tensor_copy(yt[:, 512:1024], py)
            nc.sync.dma_start(io["y"][tb * P:(tb + 1) * P, :], yt)

    if variant == "causal":
        # ---- interleaved schedule: proj(t4) ahead of attention(iq=t4) ----
        # Valid because causal attention for stripe iq only reads K/V from
        # stripes <= iq.
        emit_qk_proj(0)
        emit_v_proj(0)
        for t4 in range(1, T4):
            emit_qk_proj(t4)
            emit_attention(t4 - 1)
            emit_v_proj(t4)
            emit_wo(t4 - 1)
        emit_attention(T4 - 1)
        emit_wo(T4 - 1)
    else:
        # Non-causal: every stripe's attention reads all K/V stripes, so all
        # projections must be emitted first.
        for t4 in range(T4):
            emit_qk_proj(t4)
            emit_v_proj(t4)
        for iq in range(T4):
            emit_attention(iq)
            emit_wo(iq)


def build_program(variant: str):
    if variant in _PROG_CACHE:
        return _PROG_CACHE[variant]
    nc = bacc.Bacc("TRN2", target_bir_lowering=False, debug=False, num_devices=NCORES)
    io = {
        "qT": nc.dram_tensor("qT", [E, T], BF, kind="ExternalInput").ap(),
        "kT": nc.dram_tensor("kT", [E, T], BF, kind="ExternalInput").ap(),
        "vT": nc.dram_tensor("vT", [E, T], BF, kind="ExternalInput").ap(),
        "wq": nc.dram_tensor("wq", [2, P, KC * P], BF, kind="ExternalInput").ap(),
        "wk": nc.dram_tensor("wk", [2, P, KC * P], BF, kind="ExternalInput").ap(),
        "wv": nc.dram_tensor("wv", [P, KC * 256], BF, kind="ExternalInput").ap(),
        "woT": nc.dram_tensor("woT", [256, E], BF, kind="ExternalInput").ap(),
        "y": nc.dram_tensor("y", [T, E], BF, kind="ExternalOutput").ap(),
    }
    if variant == "causal":
        io["sc"] = nc.dram_tensor("sc", [P, 896], BF, kind="ExternalInput").ap()
    elif variant == "mask":
        io["mT"] = nc.dram_tensor("mT", [T, T], BF, kind="ExternalInput").ap()
    with tile.TileContext(nc) as tc:
        with ExitStack() as ctx:
            _emit(ctx, tc, io, variant)
    nc.compile()
    _PROG_CACHE[variant] = nc
    return nc


def make_in_maps(query, key, value, mask, Wq, Wk, Wv, Wo, variant):
    """Build the 8 per-core input maps (host-side sharding + layout prep)."""
    bf = ml_dtypes.bfloat16
    scale = np.float32(1.0 / np.sqrt(DH))
    Wq = np.asarray(Wq, np.float32) * scale
    Wk = np.asarray(Wk, np.float32)
    Wv = np.asarray(Wv, np.float32)
    Wo = np.asarray(Wo, np.float32)

    xT = {}
    for name, x in (("qT", query), ("kT", key), ("vT", value)):
        xT[name] = [np.ascontiguousarray(np.asarray(x[n], np.float32).T).astype(bf) for n in range(N)]
    mT = None
    if variant == "mask":
        mT = [np.ascontiguousarray((np.asarray(mask[n, 0]) != 0).T).astype(bf) for n in range(N)]
    sc = None
    if variant == "causal":
        sc = np.zeros((P, 896), np.float32)
        for prt in range(P):
            sc[prt, prt + 384:] = 1.0
        sc = sc.astype(bf)

    per_c4 = []
    for c4 in range(4):
        heads = [4 * c4 + i for i in range(4)]
        def swz(w):  # [E, cols] -> [128, KC*cols] partition-swizzled
            cols = w.shape[1]
            return np.ascontiguousarray(
                w.reshape(KC, P, cols).transpose(1, 0, 2).reshape(P, KC * cols))

        wq = np.stack([
            swz(np.concatenate([Wq[heads[2 * p]], Wq[heads[2 * p + 1]]], axis=1)) for p in range(2)
        ]).astype(bf)
        wk = np.stack([
            swz(np.concatenate([Wk[heads[2 * p]], Wk[heads[2 * p + 1]]], axis=1)) for p in range(2)
        ]).astype(bf)
        wv = swz(np.concatenate([Wv[h] for h in heads], axis=1)).astype(bf)
        woT = np.ascontiguousarray(Wo[:, c4 * 256:(c4 + 1) * 256].T).astype(bf)
        per_c4.append((wq, wk, wv, woT))

    in_maps = []
    for core in range(NCORES):
        n, c4 = divmod(core, 4)
        wq, wk, wv, woT = per_c4[c4]
        im = {
            "qT": xT["qT"][n], "kT": xT["kT"][n], "vT": xT["vT"][n],
            "wq": wq, "wk": wk, "wv": wv, "woT": woT,
        }
        if variant == "causal":
            im["sc"] = sc
        elif variant == "mask":
            im["mT"] = mT[n]
        in_maps.append(im)
    return in_maps


def detect_variant(mask):
    m = np.asarray(mask) != 0
    if m.all():
        return "full"
    tril = np.tril(np.ones((T, T), dtype=bool))
    if all(np.array_equal(m[n, 0], tril) for n in range(N)):
        return "causal"
    return "mask"


def kernel_run(query, key, value, mask, Wq, Wk, Wv, Wo, bo, trace=False):
    variant = detect_variant(mask)
    nc = build_program(variant)
    in_maps = make_in_maps(query, key, value, mask, Wq, Wk, Wv, Wo, variant)
    try:
        res = run_bass_kernel_spmd(nc, in_maps, core_ids=list(range(NCORES)), trace=trace)
    except ModuleNotFoundError:
        # NTFF profiling hook unavailable in this environment
        res = run_bass_kernel_spmd(nc, in_maps, core_ids=list(range(NCORES)))
    bo = np.asarray(bo, np.float32)
    out = np.empty((N, T, E), np.float32)
    for n in range(N):
        acc = np.zeros((T, E), np.float32)
        for c4 in range(4):
            acc += np.asarray(res.results[4 * n + c4]["y"], np.float32)
        out[n] = acc + bo
    return out, res


def kernel(query, key, value, mask, Wq, Wk, Wv, Wo, bo):
    out, _ = kernel_run(query, key, value, mask, Wq, Wk, Wv, Wo, bo)
    return out



# revision 31
# speedup vs baseline: 1.1917x; 1.1917x over previous
# Multi-head attention (N=2, T=2048, E=1024, H=16, DH=64) on 8 TRN2 NeuronCores.
#
# Sharding: tensor-parallel over heads x data-parallel over batch.
#   core c in 0..7 -> batch n = c // 4, heads [4*(c%4) .. 4*(c%4)+3].
# Each core computes its 4 heads' Q/K/V projections, causal attention, and a
# partial output projection (its slice of Wo). Host sums the 4 partials per
# batch and adds the bias.
#
# Device layouts (per core):
#   qT/kT/vT : [E, T] bf16   (host pre-transposes inputs; E on partitions)
#   QT/KT    : [128, T]      head-pair-stacked q^T/k^T (rows 0-63 head 2p, 64-127 head 2p+1)
#   V        : [128, 16*65]  v tiles per head with an appended ones column
#   S^T      : [Tk=128, Tq<=512] per block -> exp -> P^T in SBUF.
#   PV       : operand-swapped matmul: lhsT = P^T chunk [128k, 128q] (stationary),
#              rhs = [V|ones] [128k, 65] -> out [128 q, 65] PSUM accumulated over
#              k-blocks; col 64 is the softmax denominator. Normalize on DVE with
#              a per-partition reciprocal, PE-transpose back to [c, q] for Wo.
# Softmax skips max-subtraction: energies are pre-scaled by 1/sqrt(DH) (folded
# into Wq on host) and are O(+-10), far from fp32 exp overflow.

import os
from contextlib import ExitStack

import ml_dtypes
import numpy as np

import concourse.bass as bass
import concourse.mybir as mybir
import concourse.tile as tile
from concourse import bacc
from concourse.bass_utils import run_bass_kernel_spmd

N, T, E, H, DH = 2, 2048, 1024, 16, 64
P = 128
KC = E // P          # 8 contraction chunks for projections
TB = T // P          # 16 token blocks of 128
T4 = T // 512        # 4 token blocks of 512
HPC = 4              # heads per core
NCORES = 8
BF = mybir.dt.bfloat16
F32 = mybir.dt.float32
EXP = mybir.ActivationFunctionType.Exp

_PROG_CACHE: dict = {}


def _emit(ctx: ExitStack, tc: "tile.TileContext", io: dict, variant: str):
    nc = tc.nc
    const = ctx.enter_context(tc.tile_pool(name="const", bufs=1))
    xin = ctx.enter_context(tc.tile_pool(name="xin", bufs=1))
    proj = ctx.enter_context(tc.tile_pool(name="proj", bufs=1))
    work = ctx.enter_context(tc.tile_pool(name="work", bufs=4))
    small = ctx.enter_context(tc.tile_pool(name="small", bufs=2))
    # PSUM budget (8 banks):
    #   ps_s   : 3 bufs x [128,512] f32 = 3 banks (S blocks, one per head)
    #   ps_acc : 3 tiles (455/455/512 f32) = 3 banks -- 16 PV slots of 65 cols
    #            + 2 transpose slots in tile 2's tail
    #   ps_aux : 1 buf x [128,512] f32 = 1 bank (projections / Wo)
    ps_s = ctx.enter_context(tc.tile_pool(name="ps_s", bufs=3, space="PSUM"))
    ps_acc = ctx.enter_context(tc.tile_pool(name="ps_acc", bufs=1, space="PSUM"))
    ps_aux = ctx.enter_context(tc.tile_pool(name="ps_aux", bufs=2, space="PSUM"))

    # ---- load weights & constants (emission order ~ priority order) ----
    def load_w(srcname, p):
        w_t = const.tile([P, KC * P], BF, tag=f"{srcname}{p}", name=f"{srcname}{p}")
        nc.sync.dma_start(w_t, io[srcname][p])
        return w_t

    wq_sb = [load_w("wq", p) for p in range(2)]

    def alloc_xt(key):
        return xin.tile([P, KC * T], BF, tag=key, name=key)

    def xt_ap(t, kc, lo, hi):  # [128, hi-lo] chunk kc token-slice
        return t[:, kc * T + lo: kc * T + hi]

    def load_xt_t4(t, key, t4, eng=None, halves=False):
        eng = eng or nc.sync
        view_d = t.rearrange("p (c t) -> p c t", c=KC)
        view_s = io[key].rearrange("(c p) t -> p c t", p=P)
        if halves:
            nq = 4 if halves == "quarters" else 2
            w = KC // nq
            for h in range(nq):
                dst = view_d[:, w * h:w * h + w, t4 * 512:(t4 + 1) * 512]
                src = view_s[:, w * h:w * h + w, t4 * 512:(t4 + 1) * 512]
                eng.dma_start(dst, src)
        else:
            eng.dma_start(view_d[:, :, t4 * 512:(t4 + 1) * 512],
                          view_s[:, :, t4 * 512:(t4 + 1) * 512])

    qT_sb = alloc_xt("qT")
    kT_sb = alloc_xt("kT")
    vT_sb = alloc_xt("vT")
    wv_sb = const.tile([P, KC * 256], BF, tag="wv", name="wv")
    # Startup: spread first-stripe loads over two DGE queues so the first
    # projection groups wait on ~1 MB, not the whole input set.  sc/idn are
    # tiny and needed by the first attention block's mask/transpose.
    load_xt_t4(qT_sb, "qT", 0, eng=nc.scalar, halves="quarters")
    if variant == "causal":
        sc_sb = const.tile([P, P], BF, tag="sc", name="sc")
        nc.sync.dma_start(sc_sb, io["sc"])
    wk_sb = [load_w("wk", p) for p in range(2)]
    idn_sb = const.tile([P, P], F32, tag="idn", name="idn")
    nc.sync.dma_start(idn_sb, io["idn"])
    load_xt_t4(kT_sb, "kT", 0, halves=True)
    nc.sync.dma_start(wv_sb, io["wv"])
    load_xt_t4(vT_sb, "vT", 0, halves=True)
    woT_sb = []
    for cc in range(2):
        w_t = const.tile([P, E], BF, tag=f"woT{cc}", name=f"woT{cc}")
        nc.sync.dma_start(w_t, io["woT"][cc * P:(cc + 1) * P, :])
        woT_sb.append(w_t)
    for t4 in range(1, T4):
        load_xt_t4(qT_sb, "qT", t4)
        load_xt_t4(kT_sb, "kT", t4)
        load_xt_t4(vT_sb, "vT", t4)

    # ---- persistent SBUF tensors ----
    QT = [proj.tile([P, T], BF, tag=f"QT{p}", name=f"QT{p}") for p in range(2)]
    KT = [proj.tile([P, T], BF, tag=f"KT{p}", name=f"KT{p}") for p in range(2)]
    V4 = proj.tile([P, TB * HPC * 65], BF, tag="V4", name="V4")

    def V_ap(h, jk):  # [128, 65] rhs for the PV matmul of head h, k-block jk
        return V4[:, jk * (HPC * 65) + h * 65: jk * (HPC * 65) + h * 65 + 65]

    CT = [proj.tile([P, T], BF, tag=f"CT{p}", name=f"CT{p}") for p in range(2)]

    v3 = V4.rearrange("p (b h c) -> p b h c", b=TB, h=HPC)
    nc.vector.memset(v3[:, :, :, 64:65], 1.0)

    # PV accumulators: 16 slots of [128, 65] f32 packed into 3 bank-sized
    # tiles (7 + 7 + 2 slots); slot = p*8 + c*2 + hh.  PSUM allows only one
    # OPEN accumulation group per 2 KB bank, so per stripe each tile hosts a
    # single group: start on the tile's first emitted PV matmul, stop on its
    # last (slots inside accumulate independently via per-element has_written).
    po_t = [ps_acc.tile([P, 455], F32, tag="po0", name="po0"),
            ps_acc.tile([P, 455], F32, tag="po1", name="po1"),
            ps_acc.tile([P, 130], F32, tag="po2", name="po2")]

    def po_slot(p, c, hh):
        s = p * 8 + c * 2 + hh
        return (0, s) if s < 7 else (1, s - 7) if s < 14 else (2, s - 14)

    def po_ap(p, c, hh, width=65):
        t, i = po_slot(p, c, hh)
        return po_t[t][:, i * 65: i * 65 + width]

    def tr_ap():
        return ps_aux.tile([P, 512], F32, tag="aux", name="tr")[:, 0:P]

    # ---------------- emission units ----------------
    def emit_qk_proj(t4, which, p):
        # one [128, 512] projection group: 8 accumulating matmuls
        dst, w_sb, x_sb = ((QT[p], wq_sb[p], qT_sb), (KT[p], wk_sb[p], kT_sb))[which]
        ps = ps_aux.tile([P, 512], F32, tag="aux", name="ps_proj")
        for kc in range(KC):
            nc.tensor.matmul(
                ps,
                w_sb[:, kc * P:(kc + 1) * P],
                xt_ap(x_sb, kc, t4 * 512, (t4 + 1) * 512),
                start=(kc == 0),
                stop=(kc == KC - 1),
            )
        nc.vector.tensor_copy(dst[:, t4 * 512:(t4 + 1) * 512], ps)

    def emit_v_proj(tb):
        pv = ps_aux.tile([P, 512], F32, tag="aux", name="ps_v")
        for kc in range(KC):
            nc.tensor.matmul(
                pv[:, 0:256],
                xt_ap(vT_sb, kc, tb * P, (tb + 1) * P),
                wv_sb[:, kc * 256:(kc + 1) * 256],
                start=(kc == 0),
                stop=(kc == KC - 1),
            )
        nc.vector.tensor_copy(v3[:, tb, :, 0:64],
                              pv[:, 0:256].rearrange("p (h c) -> p h c", h=HPC))

    yt_live = {}

    drain_mode = [False]

    def emit_wo_es(tb, es):
        # half of the output projection for one finished 128-token block
        if es == 0:
            yt_live[tb] = work.tile([P, 1024], BF, tag="yt", name="yt", bufs=4)
        yt = yt_live[tb]
        if drain_mode[0]:
            py = ps_s.tile([P, 512], F32, tag="s", name="py")
        else:
            py = ps_aux.tile([P, 512], F32, tag="aux", name="py")
        for cc in range(2):
            nc.tensor.matmul(
                py,
                CT[cc][:, tb * P:(tb + 1) * P],
                woT_sb[cc][:, es * 512:(es + 1) * 512],
                start=(cc == 0),
                stop=(cc == 1),
            )
        if drain_mode[0] and es == 0:
            nc.scalar.copy(yt[:, 0:512], py)
        else:
            nc.vector.tensor_copy(yt[:, es * 512:(es + 1) * 512], py)
        if es == 1:
            nc.scalar.dma_start(io["y"][tb * P:(tb + 1) * P, :], yt)
            del yt_live[tb]

    # ---- filler management: units of PE work to interleave into attention ----
    filler: list = []          # list of (est_pe_ns, ready_ns, closure)
    est_pe = [0.0]             # cumulative emitted PE ns (attention + filler)
    est_act = [0.0]            # cumulative emitted ACT ns
    blocks_left = [1]

    def force_units(key):
        # hard-emit all filler units tagged `key` (correctness: their writes
        # must precede the attention reads that need them)
        for ent in [e for e in filler if len(e) > 4 and e[4] == key]:
            filler.remove(ent)
            ent[2]()
            est_pe[0] += ent[0]

    def pump(margin):
        # Keep emitted PE work >= emitted ACT work + margin so the in-order
        # PE stream never starves while exp runs, and drain the backlog
        # early enough that the post-attention tail is empty.  Units whose
        # input DMA has likely not landed yet (ready_ns) are deferred.
        now = max(est_pe[0], est_act[0]) + 9000.0
        pops = 0
        while filler and pops < 1 and (est_pe[0] < est_act[0] + margin
                                       or len(filler) > blocks_left[0]):
            idx = next((i for i, e in enumerate(filler)
                        if e[1] <= now and (len(e) < 4 or e[3] is None or e[3]())), None)
            if idx is None:
                break
            ent = filler.pop(idx)
            ent[2]()
            est_pe[0] += ent[0]
            pops += 1

    # Deferred transposes: emitted a couple of chunk-completions later (or
    # pulled in by the Wo unit that needs them) so the in-order PE stream
    # doesn't wait on the DVE normalize round-trip.
    pending_tr: dict = {}
    tr_q: list = []
    tr_done_at: dict = {}
    fc_ctr = [0]

    def emit_transpose(key):
        fn = pending_tr.pop(key, None)
        if fn is None:
            return
        tr_q.remove(key)
        fn()
        tr_done_at[key] = fc_ctr[0]
        est_pe[0] += 107.0

    rem_at = {1: 72, 2: 56, 3: 32}

    def finish_chunk(p, tb):
        fc_ctr[0] += 1
        # flush old transposes first so the x2 ring (bufs=4) can never
        # cycle through an unemitted PE consumer
        while len(tr_q) > 1:
            emit_transpose(tr_q[0])
        c = tb % 4
        rec = small.tile([P, 2], F32, tag="rec", name="rec", bufs=4)
        for hh in range(2):
            nc.vector.reciprocal(rec[:, hh:hh + 1], po_ap(p, c, hh)[:, 64:65])
        x2 = small.tile([P, P], F32, tag="x2", name="x2", bufs=4)
        for hh in range(2):
            nc.vector.tensor_scalar_mul(
                x2[:, hh * 64:(hh + 1) * 64],
                po_ap(p, c, hh, width=64),
                rec[:, hh:hh + 1],
            )

        def do_tr(p=p, tb=tb, x2=x2):
            tr = tr_ap()
            nc.tensor.transpose(tr, x2, idn_sb)
            nc.vector.tensor_copy(CT[p][:, tb * P:(tb + 1) * P], tr)

        key = (p, tb)
        pending_tr[key] = do_tr
        tr_q.append(key)
        if p == 1:
            def wo0(tb=tb):
                emit_transpose((0, tb))
                emit_transpose((1, tb))
                emit_wo_es(tb, 0)

            gate = rem_at.get(tb // 4 + 1, 10 ** 9)

            def wo0_ready(tb=tb, gate=gate):
                if drain_mode[0]:
                    return True
                if blocks_left[0] > gate:
                    return False
                for p_ in range(2):
                    if (p_, tb) in pending_tr:
                        return False
                    if fc_ctr[0] < tr_done_at.get((p_, tb), 0) + 2:
                        return False
                return True
            filler.append((641.0, 18000.0, wo0, wo0_ready))
            filler.append((427.0, 18000.0, lambda tb=tb: emit_wo_es(tb, 1),
                           lambda tb=tb: tb in yt_live))

    def emit_attention(iq, margin):
        njk = 4 * iq + 4 if variant == "causal" else TB
        pend = {0: [], 1: []}  # per-p [(pt, co, jk)] awaiting PV (lag 2)

        # dry-run the PV emission order to place one start/stop per po tile
        seq = []

        def chunk_range(jk):
            r = jk - 4 * iq
            cstart = max(r, 0) if variant == "causal" else 0
            return range(3, cstart - 1, -1)

        for ljk in range(njk):
            for p_ in range(2):
                if ljk >= 2:
                    for c in chunk_range(ljk - 2):
                        for hh in range(2):
                            seq.append((p_, ljk - 2, c, hh))
        for p_ in range(2):
            for jk_ in range(max(njk - 2, 0), njk):
                for c in chunk_range(jk_):
                    for hh in range(2):
                        seq.append((p_, jk_, c, hh))
        first_mm, last_mm = {}, {}
        for ent in seq:
            t, _ = po_slot(ent[0], ent[2], ent[3])
            first_mm.setdefault(t, ent)
            last_mm[t] = ent
        first_set = set(first_mm.values())
        last_set = set(last_mm.values())

        def flush(item, p, iq=iq, njk=njk):
            pt, co, jk = item
            for c in chunk_range(jk):
                jq = 4 * iq + c
                complete = (jk == jq) if variant == "causal" else (jk == njk - 1)
                for hh in range(2):
                    nc.tensor.matmul(
                        po_ap(p, c, hh),
                        pt[:, hh * 512 + c * P: hh * 512 + (c + 1) * P],
                        V_ap(2 * p + hh, jk),
                        start=((p, jk, c, hh) in first_set),
                        stop=((p, jk, c, hh) in last_set),
                        skip_group_check=True,
                    )
                est_pe[0] += 2 * 65 * 0.42
                if complete:
                    finish_chunk(p, jq)

        force_units(("q", iq))
        for jk in range(njk):
            if jk == max(4 * iq - 2, 0):
                # K/V of this stripe must be emitted before the diagonal
                # blocks (S reads KT stripe iq at jk=4iq; PV reads V there
                # too, two blocks later)
                force_units(("k", iq))
                force_units(("v", iq))
            for p in range(2):
                r = jk - 4 * iq
                co = P * r if (variant == "causal" and r >= 0) else 0
                pt = work.tile([P, 1024], BF, tag="pt", name="pt", bufs=6)
                for hh in range(2):
                    ps2 = ps_s.tile([P, 512], F32, tag="s", name="ps_s")
                    nc.tensor.matmul(
                        ps2[:, co:512],
                        KT[p][hh * 64:(hh + 1) * 64, jk * P:(jk + 1) * P],
                        QT[p][hh * 64:(hh + 1) * 64, iq * 512 + co:(iq + 1) * 512],
                        start=True,
                        stop=True,
                    )
                    est_pe[0] += (512 - co) * 0.42
                    nc.scalar.activation(pt[:, hh * 512 + co:(hh + 1) * 512],
                                         ps2[:, co:512], EXP)
                    est_act[0] += (512 - co) * 0.833 + 185
                pump(margin)
                if variant == "causal" and r >= 0:
                    for hh in range(2):
                        sl = pt[:, hh * 512 + co: hh * 512 + co + P]
                        nc.gpsimd.tensor_mul(sl, sl, sc_sb)
                elif variant == "mask":
                    mk = work.tile([P, 512], BF, tag="mk", name="mk", bufs=6)
                    nc.sync.dma_start(mk, io["mT"][jk * P:(jk + 1) * P, iq * 512:(iq + 1) * 512])
                    for hh in range(2):
                        sl = pt[:, hh * 512:(hh + 1) * 512]
                        nc.gpsimd.tensor_mul(sl, sl, mk)
                pend[p].append((pt, co, jk))
                if len(pend[p]) > 2:
                    flush(pend[p].pop(0), p)
                blocks_left[0] -= 1
                pump(margin)
        for p in range(2):
            while pend[p]:
                flush(pend[p].pop(0), p)
                pump(margin)

    # ---------------- schedule ----------------
    if variant == "causal":
        # stripe-0 projections emitted directly (nothing to overlap with yet)
        for p in range(2):
            emit_qk_proj(0, 0, p)
        for p in range(2):
            emit_qk_proj(0, 1, p)
        for tb in range(4):
            emit_v_proj(tb)
        # later-stripe projections become filler for the attention phase.
        # ready_ns ~ cumulative serial DMA transfer time when that stripe's
        # tensor has landed (weights+stripe0 ~ 14.3us, then 3.16us per load).
        for t4 in range(1, T4):
            rq = 17200.0 + (t4 - 1) * 8736.0
            rk = rq + 2912.0
            rv = rk + 2912.0
            gate = rem_at[t4]
            for p in range(2):
                filler.append((1707.0, rq, lambda t4=t4, p=p: emit_qk_proj(t4, 0, p),
                               None, ("q", t4)))
            for p in range(2):
                filler.append((1707.0, rk, lambda t4=t4, p=p: emit_qk_proj(t4, 1, p),
                               lambda g=gate: blocks_left[0] <= g, ("k", t4)))
            for tb in range(4 * t4, 4 * t4 + 4):
                filler.append((853.0, rv, lambda tb=tb: emit_v_proj(tb),
                               lambda g=gate: blocks_left[0] <= g, ("v", t4)))
        blocks_left[0] = sum(2 * (4 * iq + 4) for iq in range(T4))
        for iq in range(T4):
            emit_attention(iq, (1500.0, 1500.0, 2500.0, 3000.0)[iq])
        drain_mode[0] = True
        while filler:
            filler.pop(0)[2]()
    else:
        for t4 in range(T4):
            for p in range(2):
                emit_qk_proj(t4, 0, p)
                emit_qk_proj(t4, 1, p)
            for tb in range(4 * t4, 4 * t4 + 4):
                emit_v_proj(tb)
        blocks_left[0] = 2 * TB * T4
        for iq in range(T4):
            emit_attention(iq, 2500.0)
        drain_mode[0] = True
        while filler:
            filler.pop(0)[2]()


def build_program(variant: str):
    if variant in _PROG_CACHE:
        return _PROG_CACHE[variant]
    nc = bacc.Bacc("TRN2", target_bir_lowering=False, debug=False, num_devices=NCORES)
    io = {
        "qT": nc.dram_tensor("qT", [E, T], BF, kind="ExternalInput").ap(),
        "kT": nc.dram_tensor("kT", [E, T], BF, kind="ExternalInput").ap(),
        "vT": nc.dram_tensor("vT", [E, T], BF, kind="ExternalInput").ap(),
        "wq": nc.dram_tensor("wq", [2, P, KC * P], BF, kind="ExternalInput").ap(),
        "wk": nc.dram_tensor("wk", [2, P, KC * P], BF, kind="ExternalInput").ap(),
        "wv": nc.dram_tensor("wv", [P, KC * 256], BF, kind="ExternalInput").ap(),
        "woT": nc.dram_tensor("woT", [256, E], BF, kind="ExternalInput").ap(),
        "idn": nc.dram_tensor("idn", [P, P], F32, kind="ExternalInput").ap(),
        "y": nc.dram_tensor("y", [T, E], BF, kind="ExternalOutput").ap(),
    }
    if variant == "causal":
        io["sc"] = nc.dram_tensor("sc", [P, P], BF, kind="ExternalInput").ap()
    elif variant == "mask":
        io["mT"] = nc.dram_tensor("mT", [T, T], BF, kind="ExternalInput").ap()
    with tile.TileContext(nc) as tc:
        with ExitStack() as ctx:
            _emit(ctx, tc, io, variant)
    nc.compile()
    _PROG_CACHE[variant] = nc
    return nc


def make_in_maps(query, key, value, mask, Wq, Wk, Wv, Wo, variant):
    """Build the 8 per-core input maps (host-side sharding + layout prep)."""
    bf = ml_dtypes.bfloat16
    scale = np.float32(1.0 / np.sqrt(DH))
    Wq = np.asarray(Wq, np.float32) * scale
    Wk = np.asarray(Wk, np.float32)
    Wv = np.asarray(Wv, np.float32)
    Wo = np.asarray(Wo, np.float32)

    xT = {}
    for name, x in (("qT", query), ("kT", key), ("vT", value)):
        xT[name] = [np.ascontiguousarray(np.asarray(x[n], np.float32).T).astype(bf) for n in range(N)]
    mT = None
    if variant == "mask":
        mT = [np.ascontiguousarray((np.asarray(mask[n, 0]) != 0).T).astype(bf) for n in range(N)]
    sc = None
    if variant == "causal":
        sc = np.zeros((P, P), np.float32)
        for prt in range(P):
            sc[prt, prt:] = 1.0
        sc = sc.astype(bf)
    idn = np.eye(P, dtype=np.float32)

    per_c4 = []
    for c4 in range(4):
        heads = [4 * c4 + i for i in range(4)]
        def swz(w):  # [E, cols] -> [128, KC*cols] partition-swizzled
            cols = w.shape[1]
            return np.ascontiguousarray(
                w.reshape(KC, P, cols).transpose(1, 0, 2).reshape(P, KC * cols))

        wq = np.stack([
            swz(np.concatenate([Wq[heads[2 * p]], Wq[heads[2 * p + 1]]], axis=1)) for p in range(2)
        ]).astype(bf)
        wk = np.stack([
            swz(np.concatenate([Wk[heads[2 * p]], Wk[heads[2 * p + 1]]], axis=1)) for p in range(2)
        ]).astype(bf)
        wv = swz(np.concatenate([Wv[h] for h in heads], axis=1)).astype(bf)
        woT = np.ascontiguousarray(Wo[:, c4 * 256:(c4 + 1) * 256].T).astype(bf)
        per_c4.append((wq, wk, wv, woT))

    in_maps = []
    for core in range(NCORES):
        n, c4 = divmod(core, 4)
        wq, wk, wv, woT = per_c4[c4]
        im = {
            "qT": xT["qT"][n], "kT": xT["kT"][n], "vT": xT["vT"][n],
            "wq": wq, "wk": wk, "wv": wv, "woT": woT, "idn": idn,
        }
        if variant == "causal":
            im["sc"] = sc
        elif variant == "mask":
            im["mT"] = mT[n]
        in_maps.append(im)
    return in_maps


def detect_variant(mask):
    m = np.asarray(mask) != 0
    if m.all():
        return "full"
    tril = np.tril(np.ones((T, T), dtype=bool))
    if all(np.array_equal(m[n, 0], tril) for n in range(N)):
        return "causal"
    return "mask"


def kernel_run(query, key, value, mask, Wq, Wk, Wv, Wo, bo, trace=False):
    variant = detect_variant(mask)
    nc = build_program(variant)
    in_maps = make_in_maps(query, key, value, mask, Wq, Wk, Wv, Wo, variant)
    try:
        res = run_bass_kernel_spmd(nc, in_maps, core_ids=list(range(NCORES)), trace=trace)
    except ModuleNotFoundError:
        res = run_bass_kernel_spmd(nc, in_maps, core_ids=list(range(NCORES)))
    bo = np.asarray(bo, np.float32)
    out = np.empty((N, T, E), np.float32)
    for n in range(N):
        acc = np.zeros((T, E), np.float32)
        for c4 in range(4):
            acc += np.asarray(res.results[4 * n + c4]["y"], np.float32)
        out[n] = acc + bo
    return out, res


def kernel(query, key, value, mask, Wq, Wk, Wv, Wo, bo):
    out, _ = kernel_run(query, key, value, mask, Wq, Wk, Wv, Wo, bo)
    return out


# revision 48
# speedup vs baseline: 1.2496x; 1.0486x over previous
# Multi-head attention (N=2, T=2048, E=1024, H=16, DH=64) on 8 TRN2 NeuronCores.
#
# Sharding: tensor-parallel over heads x data-parallel over batch.
#   core c in 0..7 -> batch n = c // 4, heads [4*(c%4) .. 4*(c%4)+3].
# Each core computes its 4 heads' Q/K/V projections, causal attention, and a
# partial output projection (its slice of Wo). Host sums the 4 partials per
# batch and adds the bias.
#
# Device layouts (per core):
#   qT/kT/vT : [E, T] bf16   (host pre-transposes inputs; E on partitions)
#   QT/KT    : [128, T]      head-pair-stacked q^T/k^T (rows 0-63 head 2p, 64-127 head 2p+1)
#   V        : [128, 16*65]  v tiles per head with an appended ones column
#   S^T      : [Tk=128, Tq<=512] per block -> exp -> P^T in SBUF.
#   PV       : operand-swapped matmul: lhsT = P^T chunk [128k, 128q] (stationary),
#              rhs = [V|ones] [128k, 65] -> out [128 q, 65] PSUM accumulated over
#              k-blocks; col 64 is the softmax denominator. Normalize on DVE with
#              a per-partition reciprocal, PE-transpose back to [c, q] for Wo.
# Softmax skips max-subtraction: energies are pre-scaled by 1/sqrt(DH) (folded
# into Wq on host) and are O(+-10), far from fp32 exp overflow.

import os
from contextlib import ExitStack

import ml_dtypes
import numpy as np

import concourse.bass as bass
import concourse.mybir as mybir
import concourse.tile as tile
from concourse import bacc
from concourse.bass_utils import run_bass_kernel_spmd

N, T, E, H, DH = 2, 2048, 1024, 16, 64
P = 128
KC = E // P          # 8 contraction chunks for projections
TB = T // P          # 16 token blocks of 128
T4 = T // 512        # 4 token blocks of 512
HPC = 4              # heads per core
NCORES = 8
BF = mybir.dt.bfloat16
F32 = mybir.dt.float32
EXP = mybir.ActivationFunctionType.Exp

_PROG_CACHE: dict = {}


def _emit(ctx: ExitStack, tc: "tile.TileContext", io: dict, variant: str):
    nc = tc.nc
    const = ctx.enter_context(tc.tile_pool(name="const", bufs=1))
    xin = ctx.enter_context(tc.tile_pool(name="xin", bufs=1))
    proj = ctx.enter_context(tc.tile_pool(name="proj", bufs=1))
    work = ctx.enter_context(tc.tile_pool(name="work", bufs=4))
    small = ctx.enter_context(tc.tile_pool(name="small", bufs=2))
    # PSUM budget (8 banks):
    #   ps_s   : 3 bufs x [128,512] f32 = 3 banks (S blocks, one per head)
    #   ps_acc : 3 tiles (455/455/512 f32) = 3 banks -- 16 PV slots of 65 cols
    #            + 2 transpose slots in tile 2's tail
    #   ps_aux : 1 buf x [128,512] f32 = 1 bank (projections / Wo)
    ps_s = ctx.enter_context(tc.tile_pool(name="ps_s", bufs=3, space="PSUM"))
    ps_acc = ctx.enter_context(tc.tile_pool(name="ps_acc", bufs=1, space="PSUM"))
    ps_aux = ctx.enter_context(tc.tile_pool(name="ps_aux", bufs=2, space="PSUM"))

    # ---- load weights & constants (emission order ~ priority order) ----
    def load_w(srcname, p):
        w_t = const.tile([P, KC * P], BF, tag=f"{srcname}{p}", name=f"{srcname}{p}")
        nc.sync.dma_start(w_t, io[srcname][p])
        return w_t

    wq_sb = [load_w("wq", p) for p in range(2)]

    def alloc_xt(key):
        return xin.tile([P, KC * T], BF, tag=key, name=key)

    def xt_ap(t, kc, lo, hi):  # [128, hi-lo] chunk kc token-slice
        return t[:, kc * T + lo: kc * T + hi]

    def load_xt_t4(t, key, t4, eng=None, halves=False):
        eng = eng or nc.sync
        view_d = t.rearrange("p (c t) -> p c t", c=KC)
        view_s = io[key].rearrange("(c p) t -> p c t", p=P)
        if halves:
            nq = 4 if halves == "quarters" else 2
            w = KC // nq
            for h in range(nq):
                dst = view_d[:, w * h:w * h + w, t4 * 512:(t4 + 1) * 512]
                src = view_s[:, w * h:w * h + w, t4 * 512:(t4 + 1) * 512]
                eng.dma_start(dst, src)
        else:
            eng.dma_start(view_d[:, :, t4 * 512:(t4 + 1) * 512],
                          view_s[:, :, t4 * 512:(t4 + 1) * 512])

    qT_sb = alloc_xt("qT")
    kT_sb = alloc_xt("kT")
    vT_sb = alloc_xt("vT")
    wv_sb = const.tile([P, KC * 256], BF, tag="wv", name="wv")
    # Startup: spread first-stripe loads over two DGE queues so the first
    # projection groups wait on ~1 MB, not the whole input set.  sc/idn are
    # tiny and needed by the first attention block's mask/transpose.
    load_xt_t4(qT_sb, "qT", 0, eng=nc.scalar, halves="quarters")
    if variant == "causal":
        sc_sb = const.tile([P, P], BF, tag="sc", name="sc")
        nc.sync.dma_start(sc_sb, io["sc"])
    wk_sb = [load_w("wk", p) for p in range(2)]
    idn_sb = const.tile([P, P], F32, tag="idn", name="idn")
    nc.sync.dma_start(idn_sb, io["idn"])
    load_xt_t4(kT_sb, "kT", 0, halves=True)
    nc.sync.dma_start(wv_sb, io["wv"])
    load_xt_t4(vT_sb, "vT", 0, halves=True)
    woT_sb = []
    for cc in range(2):
        w_t = const.tile([P, E], BF, tag=f"woT{cc}", name=f"woT{cc}")
        nc.sync.dma_start(w_t, io["woT"][cc * P:(cc + 1) * P, :])
        woT_sb.append(w_t)
    for t4 in range(1, T4):
        load_xt_t4(qT_sb, "qT", t4)
        load_xt_t4(kT_sb, "kT", t4)
        load_xt_t4(vT_sb, "vT", t4)

    # ---- persistent SBUF tensors ----
    QT = [proj.tile([P, T], BF, tag=f"QT{p}", name=f"QT{p}") for p in range(2)]
    KT = [proj.tile([P, T], BF, tag=f"KT{p}", name=f"KT{p}") for p in range(2)]
    V4 = proj.tile([P, TB * HPC * 65], BF, tag="V4", name="V4")

    def V_ap(h, jk):  # [128, 65] rhs for the PV matmul of head h, k-block jk
        return V4[:, jk * (HPC * 65) + h * 65: jk * (HPC * 65) + h * 65 + 65]

    CT = [proj.tile([P, T], BF, tag=f"CT{p}", name=f"CT{p}") for p in range(2)]

    v3 = V4.rearrange("p (b h c) -> p b h c", b=TB, h=HPC)
    nc.vector.memset(v3[:, :, :, 64:65], 1.0)

    # PV accumulators: 16 slots of [128, 65] f32 packed into 3 bank-sized
    # tiles (7 + 7 + 2 slots); slot = p*8 + c*2 + hh.  PSUM allows only one
    # OPEN accumulation group per 2 KB bank, so per stripe each tile hosts a
    # single group: start on the tile's first emitted PV matmul, stop on its
    # last (slots inside accumulate independently via per-element has_written).
    po_t = [ps_acc.tile([P, 455], F32, tag="po0", name="po0"),
            ps_acc.tile([P, 455], F32, tag="po1", name="po1"),
            ps_acc.tile([P, 130], F32, tag="po2", name="po2")]

    def po_slot(p, c, hh):
        s = p * 8 + c * 2 + hh
        return (0, s) if s < 7 else (1, s - 7) if s < 14 else (2, s - 14)

    def po_ap(p, c, hh, width=65):
        t, i = po_slot(p, c, hh)
        return po_t[t][:, i * 65: i * 65 + width]

    def tr_ap():
        return ps_aux.tile([P, 512], F32, tag="aux", name="tr")[:, 0:P]

    # ---------------- emission units ----------------
    def emit_qk_proj(t4, which, p):
        # one [128, 512] projection group: 8 accumulating matmuls
        dst, w_sb, x_sb = ((QT[p], wq_sb[p], qT_sb), (KT[p], wk_sb[p], kT_sb))[which]
        ps = ps_aux.tile([P, 512], F32, tag="aux", name="ps_proj")
        for kc in range(KC):
            nc.tensor.matmul(
                ps,
                w_sb[:, kc * P:(kc + 1) * P],
                xt_ap(x_sb, kc, t4 * 512, (t4 + 1) * 512),
                start=(kc == 0),
                stop=(kc == KC - 1),
            )
        nc.vector.tensor_copy(dst[:, t4 * 512:(t4 + 1) * 512], ps)

    def emit_v_proj(tb):
        pv = ps_aux.tile([P, 512], F32, tag="aux", name="ps_v")
        for kc in range(KC):
            nc.tensor.matmul(
                pv[:, 0:256],
                xt_ap(vT_sb, kc, tb * P, (tb + 1) * P),
                wv_sb[:, kc * 256:(kc + 1) * 256],
                start=(kc == 0),
                stop=(kc == KC - 1),
            )
        nc.vector.tensor_copy(v3[:, tb, :, 0:64],
                              pv[:, 0:256].rearrange("p (h c) -> p h c", h=HPC))

    yt_live = {}

    drain_mode = [False]

    def emit_wo_es(tb, es):
        # half of the output projection for one finished 128-token block
        if es == 0:
            yt_live[tb] = work.tile([P, 1024], BF, tag="yt", name="yt", bufs=4)
        yt = yt_live[tb]
        if drain_mode[0]:
            py = ps_s.tile([P, 512], F32, tag="s", name="py")
        else:
            py = ps_aux.tile([P, 512], F32, tag="aux", name="py")
        for cc in range(2):
            nc.tensor.matmul(
                py,
                CT[cc][:, tb * P:(tb + 1) * P],
                woT_sb[cc][:, es * 512:(es + 1) * 512],
                start=(cc == 0),
                stop=(cc == 1),
            )
        if drain_mode[0] and es == 0:
            nc.scalar.copy(yt[:, 0:512], py)
        else:
            nc.vector.tensor_copy(yt[:, es * 512:(es + 1) * 512], py)
        if es == 1:
            nc.scalar.dma_start(io["y"][tb * P:(tb + 1) * P, :], yt)
            del yt_live[tb]

    # ---- filler management: units of PE work to interleave into attention ----
    filler: list = []          # list of (est_pe_ns, ready_ns, closure)
    est_pe = [0.0]             # cumulative emitted PE ns (attention + filler)
    est_act = [0.0]            # cumulative emitted ACT ns
    blocks_left = [1]

    def force_units(key):
        # hard-emit all filler units tagged `key` (correctness: their writes
        # must precede the attention reads that need them)
        for ent in [e for e in filler if len(e) > 4 and e[4] == key]:
            filler.remove(ent)
            ent[2]()
            est_pe[0] += ent[0]

    def pump(margin):
        # Keep emitted PE work >= emitted ACT work + margin so the in-order
        # PE stream never starves while exp runs, and drain the backlog
        # early enough that the post-attention tail is empty.  Units whose
        # input DMA has likely not landed yet (ready_ns) are deferred.
        now = max(est_pe[0], est_act[0]) + 9000.0
        pops = 0
        while filler and pops < 1 and (est_pe[0] < est_act[0] + margin
                                       or len(filler) > blocks_left[0]):
            idx = next((i for i, e in enumerate(filler)
                        if e[1] <= now and (len(e) < 4 or e[3] is None or e[3]())), None)
            if idx is None:
                break
            ent = filler.pop(idx)
            ent[2]()
            est_pe[0] += ent[0]
            pops += 1

    # Deferred transposes: emitted a couple of chunk-completions later (or
    # pulled in by the Wo unit that needs them) so the in-order PE stream
    # doesn't wait on the DVE normalize round-trip.
    pending_tr: dict = {}
    tr_q: list = []
    tr_done_at: dict = {}
    fc_ctr = [0]

    def emit_transpose(key):
        fn = pending_tr.pop(key, None)
        if fn is None:
            return
        tr_q.remove(key)
        fn()
        tr_done_at[key] = fc_ctr[0]
        est_pe[0] += 107.0

    rem_at = {1: 72, 2: 56, 3: 32}

    def finish_chunk(p, tb):
        fc_ctr[0] += 1
        # flush old transposes first so the x2 ring (bufs=4) can never
        # cycle through an unemitted PE consumer
        while len(tr_q) > 2:
            emit_transpose(tr_q[0])
        c = tb % 4
        rec = small.tile([P, 2], F32, tag="rec", name="rec", bufs=4)
        for hh in range(2):
            nc.vector.reciprocal(rec[:, hh:hh + 1], po_ap(p, c, hh)[:, 64:65])
        x2 = small.tile([P, P], F32, tag="x2", name="x2", bufs=4)
        for hh in range(2):
            nc.vector.tensor_scalar_mul(
                x2[:, hh * 64:(hh + 1) * 64],
                po_ap(p, c, hh, width=64),
                rec[:, hh:hh + 1],
            )

        def do_tr(p=p, tb=tb, x2=x2):
            tr = tr_ap()
            nc.tensor.transpose(tr, x2, idn_sb)
            nc.vector.tensor_copy(CT[p][:, tb * P:(tb + 1) * P], tr)

        key = (p, tb)
        pending_tr[key] = do_tr
        tr_q.append(key)
        if p == 1:
            def wo0(tb=tb):
                emit_transpose((0, tb))
                emit_transpose((1, tb))
                emit_wo_es(tb, 0)

            gate = rem_at.get(tb // 4 + 1, 10 ** 9)

            def wo0_ready(tb=tb, gate=gate):
                if drain_mode[0]:
                    return True
                if blocks_left[0] > gate:
                    return False
                for p_ in range(2):
                    if (p_, tb) in pending_tr:
                        return False
                    if fc_ctr[0] < tr_done_at.get((p_, tb), 0) + 0:
                        return False
                return True
            filler.append((641.0, 18000.0, wo0, wo0_ready))
            filler.append((427.0, 18000.0, lambda tb=tb: emit_wo_es(tb, 1),
                           lambda tb=tb: tb in yt_live))

    PRE_JK = 8   # stripe-3 k-blocks whose S/exp precompute as filler
    pt3 = {}
    if variant == "causal":
        for jk in range(PRE_JK):
            for p in range(2):
                pt3[(p, jk)] = proj.tile([P, 1024], BF, tag=f"pt3_{p}_{jk}",
                                         name=f"pt3_{p}_{jk}")

    def emit_pre_s(p, jk):
        # stripe-3 S + exp for a full block, into the persistent pt3 tile
        iq = T4 - 1
        pt = pt3[(p, jk)]
        for hh in range(2):
            ps2 = ps_s.tile([P, 512], F32, tag="s", name="ps_s")
            nc.tensor.matmul(
                ps2,
                KT[p][hh * 64:(hh + 1) * 64, jk * P:(jk + 1) * P],
                QT[p][hh * 64:(hh + 1) * 64, iq * 512:(iq + 1) * 512],
                start=True,
                stop=True,
            )
            est_pe[0] += 512 * 0.42
            nc.scalar.activation(pt[:, hh * 512:(hh + 1) * 512], ps2, EXP)
            est_act[0] += 512 * 0.833 + 185

    def emit_attention(iq, margin):
        njk = 4 * iq + 4 if variant == "causal" else TB
        pend = {0: [], 1: []}  # per-p [(pt, co, jk)] awaiting PV (lag 2)

        # dry-run the PV emission order to place one start/stop per po tile
        seq = []

        def chunk_range(jk):
            r = jk - 4 * iq
            cstart = max(r, 0) if variant == "causal" else 0
            return range(3, cstart - 1, -1)

        for ljk in range(njk):
            for p_ in range(2):
                if ljk >= 2:
                    for c in chunk_range(ljk - 2):
                        for hh in range(2):
                            seq.append((p_, ljk - 2, c, hh))
        for p_ in range(2):
            for jk_ in range(max(njk - 2, 0), njk):
                for c in chunk_range(jk_):
                    for hh in range(2):
                        seq.append((p_, jk_, c, hh))
        first_mm, last_mm = {}, {}
        for ent in seq:
            t, _ = po_slot(ent[0], ent[2], ent[3])
            first_mm.setdefault(t, ent)
            last_mm[t] = ent
        first_set = set(first_mm.values())
        last_set = set(last_mm.values())

        def flush(item, p, iq=iq, njk=njk):
            pt, co, jk = item
            for c in chunk_range(jk):
                jq = 4 * iq + c
                complete = (jk == jq) if variant == "causal" else (jk == njk - 1)
                for hh in range(2):
                    nc.tensor.matmul(
                        po_ap(p, c, hh),
                        pt[:, hh * 512 + c * P: hh * 512 + (c + 1) * P],
                        V_ap(2 * p + hh, jk),
                        start=((p, jk, c, hh) in first_set),
                        stop=((p, jk, c, hh) in last_set),
                        skip_group_check=True,
                    )
                est_pe[0] += 2 * 65 * 0.42
                if complete:
                    finish_chunk(p, jq)

        force_units(("q", iq))
        for jk in range(njk):
            if jk == max(4 * iq - 2, 0):
                # K/V of this stripe must be emitted before the diagonal
                # blocks (S reads KT stripe iq at jk=4iq; PV reads V there
                # too, two blocks later)
                force_units(("k", iq))
                force_units(("v", iq))
            for p in range(2):
                r = jk - 4 * iq
                co = P * r if (variant == "causal" and r >= 0) else 0
                if variant == "causal" and iq == T4 - 1 and jk < PRE_JK:
                    pend[p].append((pt3[(p, jk)], 0, jk))
                    if len(pend[p]) > 2:
                        flush(pend[p].pop(0), p)
                    blocks_left[0] -= 1
                    pump(margin)
                    continue
                pt = work.tile([P, 1024], BF, tag="pt", name="pt", bufs=6)
                for hh in range(2):
                    ps2 = ps_s.tile([P, 512], F32, tag="s", name="ps_s")
                    nc.tensor.matmul(
                        ps2[:, co:512],
                        KT[p][hh * 64:(hh + 1) * 64, jk * P:(jk + 1) * P],
                        QT[p][hh * 64:(hh + 1) * 64, iq * 512 + co:(iq + 1) * 512],
                        start=True,
                        stop=True,
                    )
                    est_pe[0] += (512 - co) * 0.42
                    nc.scalar.activation(pt[:, hh * 512 + co:(hh + 1) * 512],
                                         ps2[:, co:512], EXP)
                    est_act[0] += (512 - co) * 0.833 + 185
                pump(margin)
                if variant == "causal" and r >= 0:
                    for hh in range(2):
                        sl = pt[:, hh * 512 + co: hh * 512 + co + P]
                        nc.gpsimd.tensor_mul(sl, sl, sc_sb)
                elif variant == "mask":
                    mk = work.tile([P, 512], BF, tag="mk", name="mk", bufs=6)
                    nc.sync.dma_start(mk, io["mT"][jk * P:(jk + 1) * P, iq * 512:(iq + 1) * 512])
                    for hh in range(2):
                        sl = pt[:, hh * 512:(hh + 1) * 512]
                        nc.gpsimd.tensor_mul(sl, sl, mk)
                pend[p].append((pt, co, jk))
                if len(pend[p]) > 2:
                    flush(pend[p].pop(0), p)
                blocks_left[0] -= 1
                pump(margin)
        for p in range(2):
            while pend[p]:
                flush(pend[p].pop(0), p)
                pump(margin)

    # ---------------- schedule ----------------
    if variant == "causal":
        # stripe-0 projections emitted directly (nothing to overlap with yet)
        for p in range(2):
            emit_qk_proj(0, 0, p)
        for p in range(2):
            emit_qk_proj(0, 1, p)
        for tb in range(4):
            emit_v_proj(tb)
        # later-stripe projections become filler for the attention phase.
        # ready_ns ~ cumulative serial DMA transfer time when that stripe's
        # tensor has landed (weights+stripe0 ~ 14.3us, then 3.16us per load).
        for t4 in range(1, T4):
            rq = 17200.0 + (t4 - 1) * 8736.0
            rk = rq + 2912.0
            rv = rk + 2912.0
            gate = rem_at[t4]
            for p in range(2):
                filler.append((1707.0, rq, lambda t4=t4, p=p: emit_qk_proj(t4, 0, p),
                               None, ("q", t4)))
            for p in range(2):
                filler.append((1707.0, rk, lambda t4=t4, p=p: emit_qk_proj(t4, 1, p),
                               lambda g=gate: blocks_left[0] <= g, ("k", t4)))
            for tb in range(4 * t4, 4 * t4 + 4):
                filler.append((853.0, rv, lambda tb=tb: emit_v_proj(tb),
                               lambda g=gate: blocks_left[0] <= g, ("v", t4)))
        blocks_left[0] = sum(2 * (4 * iq + 4) for iq in range(T4))
        q3_done = [0]
        _orig_qk = emit_qk_proj

        def emit_qk_proj_w(t4, which, p):
            _orig_qk(t4, which, p)
            if t4 == T4 - 1 and which == 0:
                q3_done[0] += 1

        emit_qk_proj = emit_qk_proj_w  # noqa: F811 (rebind for filler closures)
        for jk in range(PRE_JK):
            for p in range(2):
                filler.append((427.0, 30000.0,
                               lambda p=p, jk=jk: emit_pre_s(p, jk),
                               lambda: q3_done[0] == 2, ("pre3",)))
        for iq in range(T4):
            if iq == T4 - 1:
                force_units(("pre3",))
            emit_attention(iq, (2000.0, 2000.0, 3000.0, 4000.0)[iq])
        drain_mode[0] = True
        while filler:
            filler.pop(0)[2]()
    else:
        for t4 in range(T4):
            for p in range(2):
                emit_qk_proj(t4, 0, p)
                emit_qk_proj(t4, 1, p)
            for tb in range(4 * t4, 4 * t4 + 4):
                emit_v_proj(tb)
        blocks_left[0] = 2 * TB * T4
        for iq in range(T4):
            emit_attention(iq, 2500.0)
        drain_mode[0] = True
        while filler:
            filler.pop(0)[2]()


def build_program(variant: str):
    if variant in _PROG_CACHE:
        return _PROG_CACHE[variant]
    nc = bacc.Bacc("TRN2", target_bir_lowering=False, debug=False, num_devices=NCORES)
    io = {
        "qT": nc.dram_tensor("qT", [E, T], BF, kind="ExternalInput").ap(),
        "kT": nc.dram_tensor("kT", [E, T], BF, kind="ExternalInput").ap(),
        "vT": nc.dram_tensor("vT", [E, T], BF, kind="ExternalInput").ap(),
        "wq": nc.dram_tensor("wq", [2, P, KC * P], BF, kind="ExternalInput").ap(),
        "wk": nc.dram_tensor("wk", [2, P, KC * P], BF, kind="ExternalInput").ap(),
        "wv": nc.dram_tensor("wv", [P, KC * 256], BF, kind="ExternalInput").ap(),
        "woT": nc.dram_tensor("woT", [256, E], BF, kind="ExternalInput").ap(),
        "idn": nc.dram_tensor("idn", [P, P], F32, kind="ExternalInput").ap(),
        "y": nc.dram_tensor("y", [T, E], BF, kind="ExternalOutput").ap(),
    }
    if variant == "causal":
        io["sc"] = nc.dram_tensor("sc", [P, P], BF, kind="ExternalInput").ap()
    elif variant == "mask":
        io["mT"] = nc.dram_tensor("mT", [T, T], BF, kind="ExternalInput").ap()
    with tile.TileContext(nc) as tc:
        with ExitStack() as ctx:
            _emit(ctx, tc, io, variant)
    nc.compile()
    _PROG_CACHE[variant] = nc
    return nc


def make_in_maps(query, key, value, mask, Wq, Wk, Wv, Wo, variant):
    """Build the 8 per-core input maps (host-side sharding + layout prep)."""
    bf = ml_dtypes.bfloat16
    scale = np.float32(1.0 / np.sqrt(DH))
    Wq = np.asarray(Wq, np.float32) * scale
    Wk = np.asarray(Wk, np.float32)
    Wv = np.asarray(Wv, np.float32)
    Wo = np.asarray(Wo, np.float32)

    xT = {}
    for name, x in (("qT", query), ("kT", key), ("vT", value)):
        xT[name] = [np.ascontiguousarray(np.asarray(x[n], np.float32).T).astype(bf) for n in range(N)]
    mT = None
    if variant == "mask":
        mT = [np.ascontiguousarray((np.asarray(mask[n, 0]) != 0).T).astype(bf) for n in range(N)]
    sc = None
    if variant == "causal":
        sc = np.zeros((P, P), np.float32)
        for prt in range(P):
            sc[prt, prt:] = 1.0
        sc = sc.astype(bf)
    idn = np.eye(P, dtype=np.float32)

    per_c4 = []
    for c4 in range(4):
        heads = [4 * c4 + i for i in range(4)]
        def swz(w):  # [E, cols] -> [128, KC*cols] partition-swizzled
            cols = w.shape[1]
            return np.ascontiguousarray(
                w.reshape(KC, P, cols).transpose(1, 0, 2).reshape(P, KC * cols))

        wq = np.stack([
            swz(np.concatenate([Wq[heads[2 * p]], Wq[heads[2 * p + 1]]], axis=1)) for p in range(2)
        ]).astype(bf)
        wk = np.stack([
            swz(np.concatenate([Wk[heads[2 * p]], Wk[heads[2 * p + 1]]], axis=1)) for p in range(2)
        ]).astype(bf)
        wv = swz(np.concatenate([Wv[h] for h in heads], axis=1)).astype(bf)
        woT = np.ascontiguousarray(Wo[:, c4 * 256:(c4 + 1) * 256].T).astype(bf)
        per_c4.append((wq, wk, wv, woT))

    in_maps = []
    for core in range(NCORES):
        n, c4 = divmod(core, 4)
        wq, wk, wv, woT = per_c4[c4]
        im = {
            "qT": xT["qT"][n], "kT": xT["kT"][n], "vT": xT["vT"][n],
            "wq": wq, "wk": wk, "wv": wv, "woT": woT, "idn": idn,
        }
        if variant == "causal":
            im["sc"] = sc
        elif variant == "mask":
            im["mT"] = mT[n]
        in_maps.append(im)
    return in_maps


def detect_variant(mask):
    m = np.asarray(mask) != 0
    if m.all():
        return "full"
    tril = np.tril(np.ones((T, T), dtype=bool))
    if all(np.array_equal(m[n, 0], tril) for n in range(N)):
        return "causal"
    return "mask"


def kernel_run(query, key, value, mask, Wq, Wk, Wv, Wo, bo, trace=False):
    variant = detect_variant(mask)
    nc = build_program(variant)
    in_maps = make_in_maps(query, key, value, mask, Wq, Wk, Wv, Wo, variant)
    try:
        res = run_bass_kernel_spmd(nc, in_maps, core_ids=list(range(NCORES)), trace=trace)
    except ModuleNotFoundError:
        res = run_bass_kernel_spmd(nc, in_maps, core_ids=list(range(NCORES)))
    bo = np.asarray(bo, np.float32)
    out = np.empty((N, T, E), np.float32)
    for n in range(N):
        acc = np.zeros((T, E), np.float32)
        for c4 in range(4):
            acc += np.asarray(res.results[4 * n + c4]["y"], np.float32)
        out[n] = acc + bo
    return out, res


def kernel(query, key, value, mask, Wq, Wk, Wv, Wo, bo):
    out, _ = kernel_run(query, key, value, mask, Wq, Wk, Wv, Wo, bo)
    return out


# revision 61
# speedup vs baseline: 1.2545x; 1.0039x over previous
# Multi-head attention (N=2, T=2048, E=1024, H=16, DH=64) on 8 TRN2 NeuronCores.
#
# Sharding: tensor-parallel over heads x data-parallel over batch.
#   core c in 0..7 -> batch n = c // 4, heads [4*(c%4) .. 4*(c%4)+3].
# Each core computes its 4 heads' Q/K/V projections, causal attention, and a
# partial output projection (its slice of Wo). Host sums the 4 partials per
# batch and adds the bias.
#
# Device layouts (per core):
#   qT/kT/vT : [E, T] bf16   (host pre-transposes inputs; E on partitions)
#   QT/KT    : [128, T]      head-pair-stacked q^T/k^T (rows 0-63 head 2p, 64-127 head 2p+1)
#   V        : [128, 16*65]  v tiles per head with an appended ones column
#   S^T      : [Tk=128, Tq<=512] per block -> exp -> P^T in SBUF.
#   PV       : operand-swapped matmul: lhsT = P^T chunk [128k, 128q] (stationary),
#              rhs = [V|ones] [128k, 65] -> out [128 q, 65] PSUM accumulated over
#              k-blocks; col 64 is the softmax denominator. Normalize on DVE with
#              a per-partition reciprocal, PE-transpose back to [c, q] for Wo.
# Softmax skips max-subtraction: energies are pre-scaled by 1/sqrt(DH) (folded
# into Wq on host) and are O(+-10), far from fp32 exp overflow.

import os
from contextlib import ExitStack

import ml_dtypes
import numpy as np

import concourse.bass as bass
import concourse.mybir as mybir
import concourse.tile as tile
from concourse import bacc
from concourse.bass_utils import run_bass_kernel_spmd

N, T, E, H, DH = 2, 2048, 1024, 16, 64
P = 128
KC = E // P          # 8 contraction chunks for projections
TB = T // P          # 16 token blocks of 128
T4 = T // 512        # 4 token blocks of 512
HPC = 4              # heads per core
NCORES = 8
BF = mybir.dt.bfloat16
F32 = mybir.dt.float32
EXP = mybir.ActivationFunctionType.Exp

_PROG_CACHE: dict = {}


def _emit(ctx: ExitStack, tc: "tile.TileContext", io: dict, variant: str):
    nc = tc.nc
    const = ctx.enter_context(tc.tile_pool(name="const", bufs=1))
    xin = ctx.enter_context(tc.tile_pool(name="xin", bufs=1))
    proj = ctx.enter_context(tc.tile_pool(name="proj", bufs=1))
    work = ctx.enter_context(tc.tile_pool(name="work", bufs=4))
    small = ctx.enter_context(tc.tile_pool(name="small", bufs=2))
    # PSUM budget (8 banks):
    #   ps_s   : 3 bufs x [128,512] f32 = 3 banks (S blocks, one per head)
    #   ps_acc : 3 tiles (455/455/512 f32) = 3 banks -- 16 PV slots of 65 cols
    #            + 2 transpose slots in tile 2's tail
    #   ps_aux : 1 buf x [128,512] f32 = 1 bank (projections / Wo)
    ps_s = ctx.enter_context(tc.tile_pool(name="ps_s", bufs=3, space="PSUM"))
    ps_acc = ctx.enter_context(tc.tile_pool(name="ps_acc", bufs=1, space="PSUM"))
    ps_aux = ctx.enter_context(tc.tile_pool(name="ps_aux", bufs=2, space="PSUM"))

    # ---- load weights & constants (emission order ~ priority order) ----
    def load_w(srcname, p):
        w_t = const.tile([P, KC * P], BF, tag=f"{srcname}{p}", name=f"{srcname}{p}")
        nc.sync.dma_start(w_t, io[srcname][p])
        return w_t

    wq_sb = [load_w("wq", p) for p in range(2)]

    def alloc_xt(key):
        return xin.tile([P, KC * T], BF, tag=key, name=key)

    def xt_ap(t, kc, lo, hi):  # [128, hi-lo] chunk kc token-slice
        return t[:, kc * T + lo: kc * T + hi]

    def load_xt_t4(t, key, t4, eng=None, halves=False):
        eng = eng or nc.sync
        view_d = t.rearrange("p (c t) -> p c t", c=KC)
        view_s = io[key].rearrange("(c p) t -> p c t", p=P)
        if halves:
            nq = 4 if halves == "quarters" else 2
            w = KC // nq
            for h in range(nq):
                dst = view_d[:, w * h:w * h + w, t4 * 512:(t4 + 1) * 512]
                src = view_s[:, w * h:w * h + w, t4 * 512:(t4 + 1) * 512]
                eng.dma_start(dst, src)
        else:
            eng.dma_start(view_d[:, :, t4 * 512:(t4 + 1) * 512],
                          view_s[:, :, t4 * 512:(t4 + 1) * 512])

    qT_sb = alloc_xt("qT")
    kT_sb = alloc_xt("kT")
    vT_sb = alloc_xt("vT")
    wv_sb = const.tile([P, KC * 256], BF, tag="wv", name="wv")
    # Startup: spread first-stripe loads over two DGE queues so the first
    # projection groups wait on ~1 MB, not the whole input set.  sc/idn are
    # tiny and needed by the first attention block's mask/transpose.
    load_xt_t4(qT_sb, "qT", 0, eng=nc.scalar, halves="quarters")
    if variant == "causal":
        sc_sb = const.tile([P, P], BF, tag="sc", name="sc")
        nc.sync.dma_start(sc_sb, io["sc"])
    wk_sb = [load_w("wk", p) for p in range(2)]
    idn_sb = const.tile([P, P], F32, tag="idn", name="idn")
    nc.sync.dma_start(idn_sb, io["idn"])
    load_xt_t4(kT_sb, "kT", 0, halves=True)
    nc.sync.dma_start(wv_sb, io["wv"])
    load_xt_t4(vT_sb, "vT", 0, halves=True)
    woT_sb = []
    for cc in range(2):
        w_t = const.tile([P, E], BF, tag=f"woT{cc}", name=f"woT{cc}")
        nc.sync.dma_start(w_t, io["woT"][cc * P:(cc + 1) * P, :])
        woT_sb.append(w_t)
    for t4 in range(1, T4):
        load_xt_t4(qT_sb, "qT", t4)
        load_xt_t4(kT_sb, "kT", t4)
        load_xt_t4(vT_sb, "vT", t4)

    # ---- persistent SBUF tensors ----
    QT = [proj.tile([P, T], BF, tag=f"QT{p}", name=f"QT{p}") for p in range(2)]
    KT = [proj.tile([P, T], BF, tag=f"KT{p}", name=f"KT{p}") for p in range(2)]
    V4 = proj.tile([P, TB * HPC * 65], BF, tag="V4", name="V4")

    def V_ap(h, jk):  # [128, 65] rhs for the PV matmul of head h, k-block jk
        return V4[:, jk * (HPC * 65) + h * 65: jk * (HPC * 65) + h * 65 + 65]

    CT = [proj.tile([P, T], BF, tag=f"CT{p}", name=f"CT{p}") for p in range(2)]

    v3 = V4.rearrange("p (b h c) -> p b h c", b=TB, h=HPC)
    nc.vector.memset(v3[:, :, :, 64:65], 1.0)

    # PV accumulators: 16 slots of [128, 65] f32 packed into 3 bank-sized
    # tiles (7 + 7 + 2 slots); slot = p*8 + c*2 + hh.  PSUM allows only one
    # OPEN accumulation group per 2 KB bank, so per stripe each tile hosts a
    # single group: start on the tile's first emitted PV matmul, stop on its
    # last (slots inside accumulate independently via per-element has_written).
    po_t = [ps_acc.tile([P, 455], F32, tag="po0", name="po0"),
            ps_acc.tile([P, 455], F32, tag="po1", name="po1"),
            ps_acc.tile([P, 130], F32, tag="po2", name="po2")]

    def po_slot(p, c, hh):
        s = p * 8 + c * 2 + hh
        return (0, s) if s < 7 else (1, s - 7) if s < 14 else (2, s - 14)

    def po_ap(p, c, hh, width=65):
        t, i = po_slot(p, c, hh)
        return po_t[t][:, i * 65: i * 65 + width]

    def tr_ap():
        return ps_aux.tile([P, 512], F32, tag="aux", name="tr")[:, 0:P]

    # ---------------- emission units ----------------
    def emit_qk_proj(t4, which, p):
        # one [128, 512] projection group: 8 accumulating matmuls
        dst, w_sb, x_sb = ((QT[p], wq_sb[p], qT_sb), (KT[p], wk_sb[p], kT_sb))[which]
        ps = ps_aux.tile([P, 512], F32, tag="aux", name="ps_proj")
        for kc in range(KC):
            nc.tensor.matmul(
                ps,
                w_sb[:, kc * P:(kc + 1) * P],
                xt_ap(x_sb, kc, t4 * 512, (t4 + 1) * 512),
                start=(kc == 0),
                stop=(kc == KC - 1),
            )
        nc.vector.tensor_copy(dst[:, t4 * 512:(t4 + 1) * 512], ps)

    def emit_v_proj(tb):
        pv = ps_aux.tile([P, 512], F32, tag="aux", name="ps_v")
        for kc in range(KC):
            nc.tensor.matmul(
                pv[:, 0:256],
                xt_ap(vT_sb, kc, tb * P, (tb + 1) * P),
                wv_sb[:, kc * 256:(kc + 1) * 256],
                start=(kc == 0),
                stop=(kc == KC - 1),
            )
        nc.vector.tensor_copy(v3[:, tb, :, 0:64],
                              pv[:, 0:256].rearrange("p (h c) -> p h c", h=HPC))

    yt_live = {}

    drain_mode = [False]

    def emit_wo_es(tb, es):
        # half of the output projection for one finished 128-token block
        if es == 0:
            yt_live[tb] = work.tile([P, 1024], BF, tag="yt", name="yt", bufs=4)
        yt = yt_live[tb]
        if drain_mode[0]:
            py = ps_s.tile([P, 512], F32, tag="s", name="py")
        else:
            py = ps_aux.tile([P, 512], F32, tag="aux", name="py")
        for cc in range(2):
            nc.tensor.matmul(
                py,
                CT[cc][:, tb * P:(tb + 1) * P],
                woT_sb[cc][:, es * 512:(es + 1) * 512],
                start=(cc == 0),
                stop=(cc == 1),
            )
        if drain_mode[0] and es == 0:
            nc.scalar.copy(yt[:, 0:512], py)
        else:
            nc.vector.tensor_copy(yt[:, es * 512:(es + 1) * 512], py)
        if es == 1:
            nc.scalar.dma_start(io["y"][tb * P:(tb + 1) * P, :], yt)
            del yt_live[tb]

    # ---- filler management: units of PE work to interleave into attention ----
    filler: list = []          # list of (est_pe_ns, ready_ns, closure)
    est_pe = [0.0]             # cumulative emitted PE ns (attention + filler)
    est_act = [0.0]            # cumulative emitted ACT ns
    blocks_left = [1]

    def force_units(key):
        # hard-emit all filler units tagged `key` (correctness: their writes
        # must precede the attention reads that need them)
        for ent in [e for e in filler if len(e) > 4 and e[4] == key]:
            filler.remove(ent)
            ent[2]()
            est_pe[0] += ent[0]

    def pump(margin):
        # Keep emitted PE work >= emitted ACT work + margin so the in-order
        # PE stream never starves while exp runs, and drain the backlog
        # early enough that the post-attention tail is empty.  Units whose
        # input DMA has likely not landed yet (ready_ns) are deferred.
        now = max(est_pe[0], est_act[0]) + 11000.0
        pops = 0
        while filler and pops < 2 and (est_pe[0] < est_act[0] + margin
                                       or len(filler) > blocks_left[0]):
            idx = next((i for i, e in enumerate(filler)
                        if e[1] <= now and (len(e) < 4 or e[3] is None or e[3]())), None)
            if idx is None:
                break
            ent = filler.pop(idx)
            ent[2]()
            est_pe[0] += ent[0]
            pops += 1

    # Deferred transposes: emitted a couple of chunk-completions later (or
    # pulled in by the Wo unit that needs them) so the in-order PE stream
    # doesn't wait on the DVE normalize round-trip.
    pending_tr: dict = {}
    tr_q: list = []
    tr_done_at: dict = {}
    fc_ctr = [0]

    def emit_transpose(key):
        fn = pending_tr.pop(key, None)
        if fn is None:
            return
        tr_q.remove(key)
        fn()
        tr_done_at[key] = fc_ctr[0]
        est_pe[0] += 107.0

    rem_at = {1: 72, 2: 56, 3: 32}

    def finish_chunk(p, tb):
        fc_ctr[0] += 1
        # flush old transposes first so the x2 ring (bufs=4) can never
        # cycle through an unemitted PE consumer
        while len(tr_q) > 2:
            emit_transpose(tr_q[0])
        c = tb % 4
        rec = small.tile([P, 2], F32, tag="rec", name="rec", bufs=4)
        for hh in range(2):
            nc.vector.reciprocal(rec[:, hh:hh + 1], po_ap(p, c, hh)[:, 64:65])
        x2 = small.tile([P, P], F32, tag="x2", name="x2", bufs=4)
        for hh in range(2):
            nc.vector.tensor_scalar_mul(
                x2[:, hh * 64:(hh + 1) * 64],
                po_ap(p, c, hh, width=64),
                rec[:, hh:hh + 1],
            )

        def do_tr(p=p, tb=tb, x2=x2):
            tr = tr_ap()
            nc.tensor.transpose(tr, x2, idn_sb)
            nc.vector.tensor_copy(CT[p][:, tb * P:(tb + 1) * P], tr)

        key = (p, tb)
        pending_tr[key] = do_tr
        tr_q.append(key)
        if p == 1:
            def wo0(tb=tb):
                emit_transpose((0, tb))
                emit_transpose((1, tb))
                emit_wo_es(tb, 0)

            gate = rem_at.get(tb // 4 + 1, 10 ** 9)

            def wo0_ready(tb=tb, gate=gate):
                if drain_mode[0]:
                    return True
                if blocks_left[0] > gate:
                    return False
                for p_ in range(2):
                    if (p_, tb) in pending_tr:
                        return False
                    if fc_ctr[0] < tr_done_at.get((p_, tb), 0) + 0:
                        return False
                return True
            filler.append((641.0, 18000.0, wo0, wo0_ready))
            filler.append((427.0, 18000.0, lambda tb=tb: emit_wo_es(tb, 1),
                           lambda tb=tb: tb in yt_live))

    PRE_JK = 8  # stripe-3 k-blocks whose S/exp precompute as filler
    pt3 = {}
    if variant == "causal":
        for jk in range(PRE_JK):
            for p in range(2):
                pt3[(p, jk)] = proj.tile([P, 1024], BF, tag=f"pt3_{p}_{jk}",
                                         name=f"pt3_{p}_{jk}")

    def emit_pre_s(p, jk):
        # stripe-3 S + exp for a full block, into the persistent pt3 tile
        iq = T4 - 1
        pt = pt3[(p, jk)]
        for hh in range(2):
            ps2 = ps_s.tile([P, 512], F32, tag="s", name="ps_s")
            nc.tensor.matmul(
                ps2,
                KT[p][hh * 64:(hh + 1) * 64, jk * P:(jk + 1) * P],
                QT[p][hh * 64:(hh + 1) * 64, iq * 512:(iq + 1) * 512],
                start=True,
                stop=True,
            )
            est_pe[0] += 512 * 0.42
            nc.scalar.activation(pt[:, hh * 512:(hh + 1) * 512], ps2, EXP)
            est_act[0] += 512 * 0.833 + 185

    def emit_attention(iq, margin):
        njk = 4 * iq + 4 if variant == "causal" else TB
        lag = 2
        pend = {0: [], 1: []}  # per-p [(pt, co, jk)] awaiting PV

        # dry-run the PV emission order to place one start/stop per po tile
        seq = []

        def chunk_range(jk):
            r = jk - 4 * iq
            cstart = max(r, 0) if variant == "causal" else 0
            return range(3, cstart - 1, -1)

        for ljk in range(njk):
            for p_ in range(2):
                if ljk >= lag:
                    for c in chunk_range(ljk - lag):
                        for hh in range(2):
                            seq.append((p_, ljk - lag, c, hh))
        for p_ in range(2):
            for jk_ in range(max(njk - lag, 0), njk):
                for c in chunk_range(jk_):
                    for hh in range(2):
                        seq.append((p_, jk_, c, hh))
        first_mm, last_mm = {}, {}
        for ent in seq:
            t, _ = po_slot(ent[0], ent[2], ent[3])
            first_mm.setdefault(t, ent)
            last_mm[t] = ent
        first_set = set(first_mm.values())
        last_set = set(last_mm.values())

        def flush(item, p, iq=iq, njk=njk):
            pt, co, jk = item
            for c in chunk_range(jk):
                jq = 4 * iq + c
                complete = (jk == jq) if variant == "causal" else (jk == njk - 1)
                for hh in range(2):
                    nc.tensor.matmul(
                        po_ap(p, c, hh),
                        pt[:, hh * 512 + c * P: hh * 512 + (c + 1) * P],
                        V_ap(2 * p + hh, jk),
                        start=((p, jk, c, hh) in first_set),
                        stop=((p, jk, c, hh) in last_set),
                        skip_group_check=True,
                    )
                est_pe[0] += 2 * 65 * 0.42
                if complete:
                    finish_chunk(p, jq)

        force_units(("q", iq))
        for jk in range(njk):
            if jk == max(4 * iq - 2, 0):
                # K/V of this stripe must be emitted before the diagonal
                # blocks (S reads KT stripe iq at jk=4iq; PV reads V there
                # too, two blocks later)
                force_units(("k", iq))
                force_units(("v", iq))
            for p in range(2):
                r = jk - 4 * iq
                co = P * r if (variant == "causal" and r >= 0) else 0
                if variant == "causal" and iq == T4 - 1 and jk < PRE_JK:
                    pend[p].append((pt3[(p, jk)], 0, jk))
                    if len(pend[p]) > lag:
                        flush(pend[p].pop(0), p)
                    blocks_left[0] -= 1
                    pump(margin)
                    continue
                pt = work.tile([P, 1024], BF, tag="pt", name="pt", bufs=6)
                for hh in range(2):
                    ps2 = ps_s.tile([P, 512], F32, tag="s", name="ps_s")
                    nc.tensor.matmul(
                        ps2[:, co:512],
                        KT[p][hh * 64:(hh + 1) * 64, jk * P:(jk + 1) * P],
                        QT[p][hh * 64:(hh + 1) * 64, iq * 512 + co:(iq + 1) * 512],
                        start=True,
                        stop=True,
                    )
                    est_pe[0] += (512 - co) * 0.42
                    nc.scalar.activation(pt[:, hh * 512 + co:(hh + 1) * 512],
                                         ps2[:, co:512], EXP)
                    est_act[0] += (512 - co) * 0.833 + 185
                pump(margin)
                if variant == "causal" and r >= 0:
                    for hh in range(2):
                        sl = pt[:, hh * 512 + co: hh * 512 + co + P]
                        nc.gpsimd.tensor_mul(sl, sl, sc_sb)
                elif variant == "mask":
                    mk = work.tile([P, 512], BF, tag="mk", name="mk", bufs=6)
                    nc.sync.dma_start(mk, io["mT"][jk * P:(jk + 1) * P, iq * 512:(iq + 1) * 512])
                    for hh in range(2):
                        sl = pt[:, hh * 512:(hh + 1) * 512]
                        nc.gpsimd.tensor_mul(sl, sl, mk)
                pend[p].append((pt, co, jk))
                if len(pend[p]) > lag:
                    flush(pend[p].pop(0), p)
                blocks_left[0] -= 1
                pump(margin)
        for p in range(2):
            while pend[p]:
                flush(pend[p].pop(0), p)
                pump(margin)

    # ---------------- schedule ----------------
    if variant == "causal":
        # stripe-0 projections emitted directly (nothing to overlap with yet)
        for p in range(2):
            emit_qk_proj(0, 0, p)
        for p in range(2):
            emit_qk_proj(0, 1, p)
        for tb in range(4):
            emit_v_proj(tb)
        # later-stripe projections become filler for the attention phase.
        # ready_ns ~ cumulative serial DMA transfer time when that stripe's
        # tensor has landed (weights+stripe0 ~ 14.3us, then 3.16us per load).
        for t4 in range(1, T4):
            rq = 17200.0 + (t4 - 1) * 8736.0
            rk = rq + 2912.0
            rv = rk + 2912.0
            gate = rem_at[t4]
            for p in range(2):
                filler.append((1707.0, rq, lambda t4=t4, p=p: emit_qk_proj(t4, 0, p),
                               None, ("q", t4)))
            for p in range(2):
                filler.append((1707.0, rk, lambda t4=t4, p=p: emit_qk_proj(t4, 1, p),
                               lambda g=gate: blocks_left[0] <= g, ("k", t4)))
            for tb in range(4 * t4, 4 * t4 + 4):
                filler.append((853.0, rv, lambda tb=tb: emit_v_proj(tb),
                               lambda g=gate: blocks_left[0] <= g, ("v", t4)))
        blocks_left[0] = sum(2 * (4 * iq + 4) for iq in range(T4))
        q3_done = [0]
        _orig_qk = emit_qk_proj

        def emit_qk_proj_w(t4, which, p):
            _orig_qk(t4, which, p)
            if t4 == T4 - 1 and which == 0:
                q3_done[0] += 1

        emit_qk_proj = emit_qk_proj_w  # noqa: F811 (rebind for filler closures)
        for jk in range(PRE_JK):
            for p in range(2):
                filler.append((427.0, 30000.0,
                               lambda p=p, jk=jk: emit_pre_s(p, jk),
                               lambda: q3_done[0] == 2, ("pre3",)))
        for iq in range(T4):
            if iq == T4 - 1:
                force_units(("pre3",))
            emit_attention(iq, (2000.0, 2000.0, 3000.0, 4000.0)[iq])
        drain_mode[0] = True
        while filler:
            filler.pop(0)[2]()
    else:
        for t4 in range(T4):
            for p in range(2):
                emit_qk_proj(t4, 0, p)
                emit_qk_proj(t4, 1, p)
            for tb in range(4 * t4, 4 * t4 + 4):
                emit_v_proj(tb)
        blocks_left[0] = 2 * TB * T4
        for iq in range(T4):
            emit_attention(iq, 2500.0)
        drain_mode[0] = True
        while filler:
            filler.pop(0)[2]()


def build_program(variant: str):
    if variant in _PROG_CACHE:
        return _PROG_CACHE[variant]
    nc = bacc.Bacc("TRN2", target_bir_lowering=False, debug=False, num_devices=NCORES)
    io = {
        "qT": nc.dram_tensor("qT", [E, T], BF, kind="ExternalInput").ap(),
        "kT": nc.dram_tensor("kT", [E, T], BF, kind="ExternalInput").ap(),
        "vT": nc.dram_tensor("vT", [E, T], BF, kind="ExternalInput").ap(),
        "wq": nc.dram_tensor("wq", [2, P, KC * P], BF, kind="ExternalInput").ap(),
        "wk": nc.dram_tensor("wk", [2, P, KC * P], BF, kind="ExternalInput").ap(),
        "wv": nc.dram_tensor("wv", [P, KC * 256], BF, kind="ExternalInput").ap(),
        "woT": nc.dram_tensor("woT", [256, E], BF, kind="ExternalInput").ap(),
        "idn": nc.dram_tensor("idn", [P, P], F32, kind="ExternalInput").ap(),
        "y": nc.dram_tensor("y", [T, E], BF, kind="ExternalOutput").ap(),
    }
    if variant == "causal":
        io["sc"] = nc.dram_tensor("sc", [P, P], BF, kind="ExternalInput").ap()
    elif variant == "mask":
        io["mT"] = nc.dram_tensor("mT", [T, T], BF, kind="ExternalInput").ap()
    with tile.TileContext(nc) as tc:
        with ExitStack() as ctx:
            _emit(ctx, tc, io, variant)
    nc.compile()
    _PROG_CACHE[variant] = nc
    return nc


def make_in_maps(query, key, value, mask, Wq, Wk, Wv, Wo, variant):
    """Build the 8 per-core input maps (host-side sharding + layout prep)."""
    bf = ml_dtypes.bfloat16
    scale = np.float32(1.0 / np.sqrt(DH))
    Wq = np.asarray(Wq, np.float32) * scale
    Wk = np.asarray(Wk, np.float32)
    Wv = np.asarray(Wv, np.float32)
    Wo = np.asarray(Wo, np.float32)

    xT = {}
    for name, x in (("qT", query), ("kT", key), ("vT", value)):
        xT[name] = [np.ascontiguousarray(np.asarray(x[n], np.float32).T).astype(bf) for n in range(N)]
    mT = None
    if variant == "mask":
        mT = [np.ascontiguousarray((np.asarray(mask[n, 0]) != 0).T).astype(bf) for n in range(N)]
    sc = None
    if variant == "causal":
        sc = np.zeros((P, P), np.float32)
        for prt in range(P):
            sc[prt, prt:] = 1.0
        sc = sc.astype(bf)
    idn = np.eye(P, dtype=np.float32)

    per_c4 = []
    for c4 in range(4):
        heads = [4 * c4 + i for i in range(4)]
        def swz(w):  # [E, cols] -> [128, KC*cols] partition-swizzled
            cols = w.shape[1]
            return np.ascontiguousarray(
                w.reshape(KC, P, cols).transpose(1, 0, 2).reshape(P, KC * cols))

        wq = np.stack([
            swz(np.concatenate([Wq[heads[2 * p]], Wq[heads[2 * p + 1]]], axis=1)) for p in range(2)
        ]).astype(bf)
        wk = np.stack([
            swz(np.concatenate([Wk[heads[2 * p]], Wk[heads[2 * p + 1]]], axis=1)) for p in range(2)
        ]).astype(bf)
        wv = swz(np.concatenate([Wv[h] for h in heads], axis=1)).astype(bf)
        woT = np.ascontiguousarray(Wo[:, c4 * 256:(c4 + 1) * 256].T).astype(bf)
        per_c4.append((wq, wk, wv, woT))

    in_maps = []
    for core in range(NCORES):
        n, c4 = divmod(core, 4)
        wq, wk, wv, woT = per_c4[c4]
        im = {
            "qT": xT["qT"][n], "kT": xT["kT"][n], "vT": xT["vT"][n],
            "wq": wq, "wk": wk, "wv": wv, "woT": woT, "idn": idn,
        }
        if variant == "causal":
            im["sc"] = sc
        elif variant == "mask":
            im["mT"] = mT[n]
        in_maps.append(im)
    return in_maps


def detect_variant(mask):
    m = np.asarray(mask) != 0
    if m.all():
        return "full"
    tril = np.tril(np.ones((T, T), dtype=bool))
    if all(np.array_equal(m[n, 0], tril) for n in range(N)):
        return "causal"
    return "mask"


def kernel_run(query, key, value, mask, Wq, Wk, Wv, Wo, bo, trace=False):
    variant = detect_variant(mask)
    nc = build_program(variant)
    in_maps = make_in_maps(query, key, value, mask, Wq, Wk, Wv, Wo, variant)
    try:
        res = run_bass_kernel_spmd(nc, in_maps, core_ids=list(range(NCORES)), trace=trace)
    except ModuleNotFoundError:
        res = run_bass_kernel_spmd(nc, in_maps, core_ids=list(range(NCORES)))
    bo = np.asarray(bo, np.float32)
    out = np.empty((N, T, E), np.float32)
    for n in range(N):
        acc = np.zeros((T, E), np.float32)
        for c4 in range(4):
            acc += np.asarray(res.results[4 * n + c4]["y"], np.float32)
        out[n] = acc + bo
    return out, res


def kernel(query, key, value, mask, Wq, Wk, Wv, Wo, bo):
    out, _ = kernel_run(query, key, value, mask, Wq, Wk, Wv, Wo, bo)
    return out


# revision 78
# speedup vs baseline: 1.2814x; 1.0215x over previous
# Multi-head attention (N=2, T=2048, E=1024, H=16, DH=64) on 8 TRN2 NeuronCores.
#
# Sharding: tensor-parallel over heads x data-parallel over batch.
#   core c in 0..7 -> batch n = c // 4, heads [4*(c%4) .. 4*(c%4)+3].
# Each core computes its 4 heads' Q/K/V projections, causal attention, and a
# partial output projection (its slice of Wo). Host sums the 4 partials per
# batch and adds the bias.
#
# Device layouts (per core):
#   qT/kT/vT : [E, T] bf16   (host pre-transposes inputs; E on partitions)
#   QT/KT    : [128, T]      head-pair-stacked q^T/k^T (rows 0-63 head 2p, 64-127 head 2p+1)
#   V        : [128, 16*65]  v tiles per head with an appended ones column
#   S^T      : [Tk=128, Tq<=512] per block -> exp -> P^T in SBUF.
#   PV       : operand-swapped matmul: lhsT = P^T chunk [128k, 128q] (stationary),
#              rhs = [V|ones] [128k, 65] -> out [128 q, 65] PSUM accumulated over
#              k-blocks; col 64 is the softmax denominator. Normalize on DVE with
#              a per-partition reciprocal, PE-transpose back to [c, q] for Wo.
# Softmax skips max-subtraction: energies are pre-scaled by 1/sqrt(DH) (folded
# into Wq on host) and are O(+-10), far from fp32 exp overflow.

import os
from contextlib import ExitStack

import ml_dtypes
import numpy as np

import concourse.bass as bass
import concourse.mybir as mybir
import concourse.tile as tile
from concourse import bacc
from concourse.bass_utils import run_bass_kernel_spmd

N, T, E, H, DH = 2, 2048, 1024, 16, 64
P = 128
KC = E // P          # 8 contraction chunks for projections
TB = T // P          # 16 token blocks of 128
T4 = T // 512        # 4 token blocks of 512
HPC = 4              # heads per core
NCORES = 8
BF = mybir.dt.bfloat16
F32 = mybir.dt.float32
EXP = mybir.ActivationFunctionType.Exp

_PROG_CACHE: dict = {}


def _emit(ctx: ExitStack, tc: "tile.TileContext", io: dict, variant: str):
    nc = tc.nc
    const = ctx.enter_context(tc.tile_pool(name="const", bufs=1))
    xin = ctx.enter_context(tc.tile_pool(name="xin", bufs=1))
    proj = ctx.enter_context(tc.tile_pool(name="proj", bufs=1))
    work = ctx.enter_context(tc.tile_pool(name="work", bufs=4))
    small = ctx.enter_context(tc.tile_pool(name="small", bufs=2))
    # PSUM budget (8 banks):
    #   ps_s   : 3 bufs x [128,512] f32 = 3 banks (S blocks, one per head)
    #   ps_acc : 3 tiles (455/455/512 f32) = 3 banks -- 16 PV slots of 65 cols
    #            + 2 transpose slots in tile 2's tail
    #   ps_aux : 1 buf x [128,512] f32 = 1 bank (projections / Wo)
    ps_s = ctx.enter_context(tc.tile_pool(name="ps_s", bufs=3, space="PSUM"))
    ps_acc = ctx.enter_context(tc.tile_pool(name="ps_acc", bufs=1, space="PSUM"))
    ps_aux = ctx.enter_context(tc.tile_pool(name="ps_aux", bufs=2, space="PSUM"))

    # ---- load weights & constants (emission order ~ priority order) ----
    def load_w(srcname, p):
        w_t = const.tile([P, KC * P], BF, tag=f"{srcname}{p}", name=f"{srcname}{p}")
        nc.sync.dma_start(w_t, io[srcname][p])
        return w_t

    wq_sb = [load_w("wq", p) for p in range(2)]

    def alloc_xt(key):
        return xin.tile([P, KC * T], BF, tag=key, name=key)

    def xt_ap(t, kc, lo, hi):  # [128, hi-lo] chunk kc token-slice
        return t[:, kc * T + lo: kc * T + hi]

    def load_xt_t4(t, key, t4, eng=None, halves=False):
        eng = eng or nc.sync
        view_d = t.rearrange("p (c t) -> p c t", c=KC)
        view_s = io[key].rearrange("(c p) t -> p c t", p=P)
        if halves:
            nq = 4 if halves == "quarters" else 2
            w = KC // nq
            for h in range(nq):
                dst = view_d[:, w * h:w * h + w, t4 * 512:(t4 + 1) * 512]
                src = view_s[:, w * h:w * h + w, t4 * 512:(t4 + 1) * 512]
                eng.dma_start(dst, src)
        else:
            eng.dma_start(view_d[:, :, t4 * 512:(t4 + 1) * 512],
                          view_s[:, :, t4 * 512:(t4 + 1) * 512])

    qT_sb = alloc_xt("qT")
    kT_sb = alloc_xt("kT")
    vT_sb = alloc_xt("vT")
    wv_sb = const.tile([P, KC * 256], BF, tag="wv", name="wv")
    # Startup: spread first-stripe loads over two DGE queues so the first
    # projection groups wait on ~1 MB, not the whole input set.  sc/idn are
    # tiny and needed by the first attention block's mask/transpose.
    load_xt_t4(qT_sb, "qT", 0, eng=nc.scalar, halves="quarters")
    if variant == "causal":
        sc_sb = const.tile([P, P], BF, tag="sc", name="sc")
        nc.sync.dma_start(sc_sb, io["sc"])
    wk_sb = [load_w("wk", p) for p in range(2)]
    idn_sb = const.tile([P, P], BF, tag="idn", name="idn")
    nc.sync.dma_start(idn_sb, io["idn"])
    load_xt_t4(kT_sb, "kT", 0, halves=True)
    nc.sync.dma_start(wv_sb, io["wv"])
    load_xt_t4(vT_sb, "vT", 0, halves=True)
    woT_sb = []
    for cc in range(2):
        w_t = const.tile([P, E], BF, tag=f"woT{cc}", name=f"woT{cc}")
        nc.sync.dma_start(w_t, io["woT"][cc * P:(cc + 1) * P, :])
        woT_sb.append(w_t)
    for t4 in range(1, T4):
        load_xt_t4(qT_sb, "qT", t4)
        load_xt_t4(kT_sb, "kT", t4)
        load_xt_t4(vT_sb, "vT", t4)

    # ---- persistent SBUF tensors ----
    QT = [proj.tile([P, T], BF, tag=f"QT{p}", name=f"QT{p}") for p in range(2)]
    KT = [proj.tile([P, T], BF, tag=f"KT{p}", name=f"KT{p}") for p in range(2)]
    V4 = proj.tile([P, TB * HPC * 65], BF, tag="V4", name="V4")

    def V_ap(h, jk):  # [128, 65] rhs for the PV matmul of head h, k-block jk
        return V4[:, jk * (HPC * 65) + h * 65: jk * (HPC * 65) + h * 65 + 65]

    CT = [proj.tile([P, T], BF, tag=f"CT{p}", name=f"CT{p}") for p in range(2)]

    v3 = V4.rearrange("p (b h c) -> p b h c", b=TB, h=HPC)
    nc.vector.memset(v3[:, :, :, 64:65], 1.0)

    # PV accumulators: 16 slots of [128, 65] f32 packed into 3 bank-sized
    # tiles (7 + 7 + 2 slots); slot = p*8 + c*2 + hh.  PSUM allows only one
    # OPEN accumulation group per 2 KB bank, so per stripe each tile hosts a
    # single group: start on the tile's first emitted PV matmul, stop on its
    # last (slots inside accumulate independently via per-element has_written).
    po_t = [ps_acc.tile([P, 455], F32, tag="po0", name="po0"),
            ps_acc.tile([P, 455], F32, tag="po1", name="po1"),
            ps_acc.tile([P, 130], F32, tag="po2", name="po2")]

    def po_slot(p, c, hh):
        s = p * 8 + c * 2 + hh
        return (0, s) if s < 7 else (1, s - 7) if s < 14 else (2, s - 14)

    def po_ap(p, c, hh, width=65):
        t, i = po_slot(p, c, hh)
        return po_t[t][:, i * 65: i * 65 + width]

    def tr_ap():
        return ps_aux.tile([P, 512], F32, tag="aux", name="tr")[:, 0:64].bitcast(BF)

    # ---------------- emission units ----------------
    def emit_qk_proj(t4, which, p):
        # one [128, 512] projection group: 8 accumulating matmuls
        dst, w_sb, x_sb = ((QT[p], wq_sb[p], qT_sb), (KT[p], wk_sb[p], kT_sb))[which]
        ps = ps_aux.tile([P, 512], F32, tag="aux", name="ps_proj")
        for kc in range(KC):
            nc.tensor.matmul(
                ps,
                w_sb[:, kc * P:(kc + 1) * P],
                xt_ap(x_sb, kc, t4 * 512, (t4 + 1) * 512),
                start=(kc == 0),
                stop=(kc == KC - 1),
            )
        nc.vector.tensor_copy(dst[:, t4 * 512:(t4 + 1) * 512], ps)

    def emit_v_proj(tb):
        pv = ps_aux.tile([P, 512], F32, tag="aux", name="ps_v")
        for kc in range(KC):
            nc.tensor.matmul(
                pv[:, 0:256],
                xt_ap(vT_sb, kc, tb * P, (tb + 1) * P),
                wv_sb[:, kc * 256:(kc + 1) * 256],
                start=(kc == 0),
                stop=(kc == KC - 1),
            )
        nc.vector.tensor_copy(v3[:, tb, :, 0:64],
                              pv[:, 0:256].rearrange("p (h c) -> p h c", h=HPC))

    yt_live = {}

    drain_mode = [False]

    def emit_wo_es(tb, es):
        # half of the output projection for one finished 128-token block
        if es == 0:
            yt_live[tb] = work.tile([P, 1024], BF, tag="yt", name="yt", bufs=4)
        yt = yt_live[tb]
        if drain_mode[0]:
            py = ps_s.tile([P, 512], F32, tag="s", name="py")
        else:
            py = ps_aux.tile([P, 512], F32, tag="aux", name="py")
        for cc in range(2):
            nc.tensor.matmul(
                py,
                CT[cc][:, tb * P:(tb + 1) * P],
                woT_sb[cc][:, es * 512:(es + 1) * 512],
                start=(cc == 0),
                stop=(cc == 1),
            )
        if drain_mode[0] and es == 0:
            nc.scalar.copy(yt[:, 0:512], py)
        else:
            nc.vector.tensor_copy(yt[:, es * 512:(es + 1) * 512], py)
        nc.scalar.dma_start(io["y"][tb * P:(tb + 1) * P, es * 512:(es + 1) * 512],
                            yt[:, es * 512:(es + 1) * 512])
        if es == 1:
            del yt_live[tb]

    # ---- filler management: units of PE work to interleave into attention ----
    filler: list = []          # list of (est_pe_ns, ready_ns, closure)
    est_pe = [0.0]             # cumulative emitted PE ns (attention + filler)
    est_act = [0.0]            # cumulative emitted ACT ns
    blocks_left = [1]

    def force_units(key):
        # hard-emit all filler units tagged `key` (correctness: their writes
        # must precede the attention reads that need them)
        for ent in [e for e in filler if len(e) > 4 and e[4] == key]:
            filler.remove(ent)
            ent[2]()
            est_pe[0] += ent[0]

    def pump(margin):
        # Keep emitted PE work >= emitted ACT work + margin so the in-order
        # PE stream never starves while exp runs, and drain the backlog
        # early enough that the post-attention tail is empty.  Units whose
        # input DMA has likely not landed yet (ready_ns) are deferred.
        now = max(est_pe[0], est_act[0]) + 10500.0
        pops = 0
        while filler and pops < 2 and (est_pe[0] < est_act[0] + margin
                                       or len(filler) > blocks_left[0]):
            idx = next((i for i, e in enumerate(filler)
                        if e[1] <= now and (len(e) < 4 or e[3] is None or e[3]())), None)
            if idx is None:
                break
            ent = filler.pop(idx)
            ent[2]()
            est_pe[0] += ent[0]
            pops += 1

    # Deferred transposes: emitted a couple of chunk-completions later (or
    # pulled in by the Wo unit that needs them) so the in-order PE stream
    # doesn't wait on the DVE normalize round-trip.
    pending_tr: dict = {}
    tr_q: list = []
    tr_done_at: dict = {}
    fc_ctr = [0]

    def emit_transpose(key):
        fn = pending_tr.pop(key, None)
        if fn is None:
            return
        tr_q.remove(key)
        fn()
        tr_done_at[key] = fc_ctr[0]
        est_pe[0] += 107.0

    rem_at = {1: 72, 2: 56, 3: 32}

    def finish_chunk(p, tb):
        fc_ctr[0] += 1
        # flush old transposes first so the x2 ring (bufs=4) can never
        # cycle through an unemitted PE consumer
        while len(tr_q) > 4:
            emit_transpose(tr_q[0])
        c = tb % 4
        rec = small.tile([P, 2], F32, tag="rec", name="rec", bufs=6)
        for hh in range(2):
            nc.vector.reciprocal(rec[:, hh:hh + 1], po_ap(p, c, hh)[:, 64:65])
        x2 = small.tile([P, P], F32, tag="x2", name="x2", bufs=6)
        for hh in range(2):
            nc.vector.tensor_scalar_mul(
                x2[:, hh * 64:(hh + 1) * 64],
                po_ap(p, c, hh, width=64),
                rec[:, hh:hh + 1],
            )

        def do_tr(p=p, tb=tb, x2=x2):
            tr = tr_ap()
            nc.tensor.transpose(tr, x2, idn_sb)
            nc.vector.tensor_copy(CT[p][:, tb * P:(tb + 1) * P], tr)

        key = (p, tb)
        pending_tr[key] = do_tr
        tr_q.append(key)
        if p == 1:
            def wo0(tb=tb):
                emit_transpose((0, tb))
                emit_transpose((1, tb))
                emit_wo_es(tb, 0)

            gate = rem_at.get(tb // 4 + 1, 10 ** 9)

            def wo0_ready(tb=tb, gate=gate):
                if drain_mode[0]:
                    return True

                for p_ in range(2):
                    if (p_, tb) in pending_tr:
                        return False
                    if fc_ctr[0] < tr_done_at.get((p_, tb), 0) + 0:
                        return False
                return True
            filler.append((641.0, 18000.0, wo0, wo0_ready))
            filler.append((427.0, 18000.0, lambda tb=tb: emit_wo_es(tb, 1),
                           lambda tb=tb: tb in yt_live))

    PRE_JK = 8  # stripe-3 k-blocks whose S/exp precompute as filler
    pt3 = {}
    if variant == "causal":
        for jk in range(PRE_JK):
            for p in range(2):
                pt3[(p, jk)] = proj.tile([P, 1024], BF, tag=f"pt3_{p}_{jk}",
                                         name=f"pt3_{p}_{jk}")

    def emit_pre_s(p, jk):
        # stripe-3 S + exp for a full block, into the persistent pt3 tile
        iq = T4 - 1
        pt = pt3[(p, jk)]
        for hh in range(2):
            ps2 = ps_s.tile([P, 512], F32, tag="s", name="ps_s")
            nc.tensor.matmul(
                ps2,
                KT[p][hh * 64:(hh + 1) * 64, jk * P:(jk + 1) * P],
                QT[p][hh * 64:(hh + 1) * 64, iq * 512:(iq + 1) * 512],
                start=True,
                stop=True,
            )
            est_pe[0] += 512 * 0.42
            nc.scalar.activation(pt[:, hh * 512:(hh + 1) * 512], ps2, EXP)
            est_act[0] += 512 * 0.833 + 185

    def emit_attention(iq, margin):
        njk = 4 * iq + 4 if variant == "causal" else TB
        lag = 2
        pend = {0: [], 1: []}  # per-p [(pt, co, jk)] awaiting PV

        # dry-run the PV emission order to place one start/stop per po tile
        seq = []

        def chunk_range(jk):
            r = jk - 4 * iq
            cstart = max(r, 0) if variant == "causal" else 0
            return range(3, cstart - 1, -1)

        for ljk in range(njk):
            for p_ in range(2):
                if ljk >= lag:
                    for c in chunk_range(ljk - lag):
                        for hh in range(2):
                            seq.append((p_, ljk - lag, c, hh))
        for p_ in range(2):
            for jk_ in range(max(njk - lag, 0), njk):
                for c in chunk_range(jk_):
                    for hh in range(2):
                        seq.append((p_, jk_, c, hh))
        first_mm, last_mm = {}, {}
        for ent in seq:
            t, _ = po_slot(ent[0], ent[2], ent[3])
            first_mm.setdefault(t, ent)
            last_mm[t] = ent
        first_set = set(first_mm.values())
        last_set = set(last_mm.values())

        def flush(item, p, iq=iq, njk=njk):
            pt, co, jk = item
            for c in chunk_range(jk):
                jq = 4 * iq + c
                complete = (jk == jq) if variant == "causal" else (jk == njk - 1)
                for hh in range(2):
                    nc.tensor.matmul(
                        po_ap(p, c, hh),
                        pt[:, hh * 512 + c * P: hh * 512 + (c + 1) * P],
                        V_ap(2 * p + hh, jk),
                        start=((p, jk, c, hh) in first_set),
                        stop=((p, jk, c, hh) in last_set),
                        skip_group_check=True,
                    )
                est_pe[0] += 2 * 65 * 0.42
                if complete:
                    finish_chunk(p, jq)

        force_units(("q", iq))
        for jk in range(njk):
            if jk == max(4 * iq - 2, 0):
                # K/V of this stripe must be emitted before the diagonal
                # blocks (S reads KT stripe iq at jk=4iq; PV reads V there
                # too, two blocks later)
                force_units(("k", iq))
                force_units(("v", iq))
            for p in range(2):
                r = jk - 4 * iq
                co = P * r if (variant == "causal" and r >= 0) else 0
                if variant == "causal" and iq == T4 - 1 and jk < PRE_JK:
                    pend[p].append((pt3[(p, jk)], 0, jk))
                    if len(pend[p]) > lag:
                        flush(pend[p].pop(0), p)
                    blocks_left[0] -= 1
                    pump(margin)
                    continue
                pt = work.tile([P, 1024], BF, tag="pt", name="pt", bufs=6)
                for hh in range(2):
                    ps2 = ps_s.tile([P, 512], F32, tag="s", name="ps_s")
                    nc.tensor.matmul(
                        ps2[:, co:512],
                        KT[p][hh * 64:(hh + 1) * 64, jk * P:(jk + 1) * P],
                        QT[p][hh * 64:(hh + 1) * 64, iq * 512 + co:(iq + 1) * 512],
                        start=True,
                        stop=True,
                    )
                    est_pe[0] += (512 - co) * 0.42
                    nc.scalar.activation(pt[:, hh * 512 + co:(hh + 1) * 512],
                                         ps2[:, co:512], EXP)
                    est_act[0] += (512 - co) * 0.833 + 185
                pump(margin)
                if variant == "causal" and r >= 0:
                    for hh in range(2):
                        sl = pt[:, hh * 512 + co: hh * 512 + co + P]
                        nc.gpsimd.tensor_mul(sl, sl, sc_sb)
                elif variant == "mask":
                    mk = work.tile([P, 512], BF, tag="mk", name="mk", bufs=6)
                    nc.sync.dma_start(mk, io["mT"][jk * P:(jk + 1) * P, iq * 512:(iq + 1) * 512])
                    for hh in range(2):
                        sl = pt[:, hh * 512:(hh + 1) * 512]
                        nc.gpsimd.tensor_mul(sl, sl, mk)
                pend[p].append((pt, co, jk))
                if len(pend[p]) > lag:
                    flush(pend[p].pop(0), p)
                blocks_left[0] -= 1
                pump(margin)
        for p in range(2):
            while pend[p]:
                flush(pend[p].pop(0), p)
                pump(margin)

    # ---------------- schedule ----------------
    if variant == "causal":
        # stripe-0 projections emitted directly (nothing to overlap with yet)
        for p in range(2):
            emit_qk_proj(0, 0, p)
        for p in range(2):
            emit_qk_proj(0, 1, p)
        for tb in range(4):
            emit_v_proj(tb)
        # later-stripe projections become filler for the attention phase.
        # ready_ns ~ cumulative serial DMA transfer time when that stripe's
        # tensor has landed (weights+stripe0 ~ 14.3us, then 3.16us per load).
        for t4 in range(1, T4):
            rq = 16200.0 + (t4 - 1) * 8736.0
            rk = rq + 2912.0
            rv = rk + 2912.0
            gate = rem_at[t4]
            for p in range(2):
                filler.append((1707.0, rq, lambda t4=t4, p=p: emit_qk_proj(t4, 0, p),
                               None, ("q", t4)))
            for p in range(2):
                filler.append((1707.0, rk, lambda t4=t4, p=p: emit_qk_proj(t4, 1, p),
                               lambda g=gate: blocks_left[0] <= g, ("k", t4)))
            for tb in range(4 * t4, 4 * t4 + 4):
                filler.append((853.0, rv, lambda tb=tb: emit_v_proj(tb),
                               lambda g=gate: blocks_left[0] <= g, ("v", t4)))
        blocks_left[0] = sum(2 * (4 * iq + 4) for iq in range(T4))
        q3_done = [0]
        _orig_qk = emit_qk_proj

        def emit_qk_proj_w(t4, which, p):
            _orig_qk(t4, which, p)
            if t4 == T4 - 1 and which == 0:
                q3_done[0] += 1

        emit_qk_proj = emit_qk_proj_w  # noqa: F811 (rebind for filler closures)
        for jk in range(PRE_JK):
            for p in range(2):
                filler.append((427.0, 30000.0,
                               lambda p=p, jk=jk: emit_pre_s(p, jk),
                               lambda: q3_done[0] == 2, ("pre3",)))
        for iq in range(T4):
            if iq == T4 - 1:
                force_units(("pre3",))
            emit_attention(iq, (2000.0, 2000.0, 1600.0, 3600.0)[iq])
        drain_mode[0] = True
        while filler:
            filler.pop(0)[2]()
    else:
        for t4 in range(T4):
            for p in range(2):
                emit_qk_proj(t4, 0, p)
                emit_qk_proj(t4, 1, p)
            for tb in range(4 * t4, 4 * t4 + 4):
                emit_v_proj(tb)
        blocks_left[0] = 2 * TB * T4
        for iq in range(T4):
            emit_attention(iq, 2500.0)
        drain_mode[0] = True
        while filler:
            filler.pop(0)[2]()


def build_program(variant: str):
    if variant in _PROG_CACHE:
        return _PROG_CACHE[variant]
    nc = bacc.Bacc("TRN2", target_bir_lowering=False, debug=False, num_devices=NCORES)
    io = {
        "qT": nc.dram_tensor("qT", [E, T], BF, kind="ExternalInput").ap(),
        "kT": nc.dram_tensor("kT", [E, T], BF, kind="ExternalInput").ap(),
        "vT": nc.dram_tensor("vT", [E, T], BF, kind="ExternalInput").ap(),
        "wq": nc.dram_tensor("wq", [2, P, KC * P], BF, kind="ExternalInput").ap(),
        "wk": nc.dram_tensor("wk", [2, P, KC * P], BF, kind="ExternalInput").ap(),
        "wv": nc.dram_tensor("wv", [P, KC * 256], BF, kind="ExternalInput").ap(),
        "woT": nc.dram_tensor("woT", [256, E], BF, kind="ExternalInput").ap(),
        "idn": nc.dram_tensor("idn", [P, P], BF, kind="ExternalInput").ap(),
        "y": nc.dram_tensor("y", [T, E], BF, kind="ExternalOutput").ap(),
    }
    if variant == "causal":
        io["sc"] = nc.dram_tensor("sc", [P, P], BF, kind="ExternalInput").ap()
    elif variant == "mask":
        io["mT"] = nc.dram_tensor("mT", [T, T], BF, kind="ExternalInput").ap()
    with tile.TileContext(nc) as tc:
        with ExitStack() as ctx:
            _emit(ctx, tc, io, variant)
    nc.compile()
    _PROG_CACHE[variant] = nc
    return nc


def make_in_maps(query, key, value, mask, Wq, Wk, Wv, Wo, variant):
    """Build the 8 per-core input maps (host-side sharding + layout prep)."""
    bf = ml_dtypes.bfloat16
    scale = np.float32(1.0 / np.sqrt(DH))
    Wq = np.asarray(Wq, np.float32) * scale
    Wk = np.asarray(Wk, np.float32)
    Wv = np.asarray(Wv, np.float32)
    Wo = np.asarray(Wo, np.float32)

    xT = {}
    for name, x in (("qT", query), ("kT", key), ("vT", value)):
        xT[name] = [np.ascontiguousarray(np.asarray(x[n], np.float32).T).astype(bf) for n in range(N)]
    mT = None
    if variant == "mask":
        mT = [np.ascontiguousarray((np.asarray(mask[n, 0]) != 0).T).astype(bf) for n in range(N)]
    sc = None
    if variant == "causal":
        sc = np.zeros((P, P), np.float32)
        for prt in range(P):
            sc[prt, prt:] = 1.0
        sc = sc.astype(bf)
    idn = np.eye(P, dtype=np.float32).astype(bf)

    per_c4 = []
    for c4 in range(4):
        heads = [4 * c4 + i for i in range(4)]
        def swz(w):  # [E, cols] -> [128, KC*cols] partition-swizzled
            cols = w.shape[1]
            return np.ascontiguousarray(
                w.reshape(KC, P, cols).transpose(1, 0, 2).reshape(P, KC * cols))

        wq = np.stack([
            swz(np.concatenate([Wq[heads[2 * p]], Wq[heads[2 * p + 1]]], axis=1)) for p in range(2)
        ]).astype(bf)
        wk = np.stack([
            swz(np.concatenate([Wk[heads[2 * p]], Wk[heads[2 * p + 1]]], axis=1)) for p in range(2)
        ]).astype(bf)
        wv = swz(np.concatenate([Wv[h] for h in heads], axis=1)).astype(bf)
        woT = np.ascontiguousarray(Wo[:, c4 * 256:(c4 + 1) * 256].T).astype(bf)
        per_c4.append((wq, wk, wv, woT))

    in_maps = []
    for core in range(NCORES):
        n, c4 = divmod(core, 4)
        wq, wk, wv, woT = per_c4[c4]
        im = {
            "qT": xT["qT"][n], "kT": xT["kT"][n], "vT": xT["vT"][n],
            "wq": wq, "wk": wk, "wv": wv, "woT": woT, "idn": idn,
        }
        if variant == "causal":
            im["sc"] = sc
        elif variant == "mask":
            im["mT"] = mT[n]
        in_maps.append(im)
    return in_maps


def detect_variant(mask):
    m = np.asarray(mask) != 0
    if m.all():
        return "full"
    tril = np.tril(np.ones((T, T), dtype=bool))
    if all(np.array_equal(m[n, 0], tril) for n in range(N)):
        return "causal"
    return "mask"


def kernel_run(query, key, value, mask, Wq, Wk, Wv, Wo, bo, trace=False):
    variant = detect_variant(mask)
    nc = build_program(variant)
    in_maps = make_in_maps(query, key, value, mask, Wq, Wk, Wv, Wo, variant)
    try:
        res = run_bass_kernel_spmd(nc, in_maps, core_ids=list(range(NCORES)), trace=trace)
    except ModuleNotFoundError:
        res = run_bass_kernel_spmd(nc, in_maps, core_ids=list(range(NCORES)))
    bo = np.asarray(bo, np.float32)
    out = np.empty((N, T, E), np.float32)
    for n in range(N):
        acc = np.zeros((T, E), np.float32)
        for c4 in range(4):
            acc += np.asarray(res.results[4 * n + c4]["y"], np.float32)
        out[n] = acc + bo
    return out, res


def kernel(query, key, value, mask, Wq, Wk, Wv, Wo, bo):
    out, _ = kernel_run(query, key, value, mask, Wq, Wk, Wv, Wo, bo)
    return out


# revision 85
# speedup vs baseline: 1.2849x; 1.0027x over previous
# Multi-head attention (N=2, T=2048, E=1024, H=16, DH=64) on 8 TRN2 NeuronCores.
#
# Sharding: tensor-parallel over heads x data-parallel over batch.
#   core c in 0..7 -> batch n = c // 4, heads [4*(c%4) .. 4*(c%4)+3].
# Each core computes its 4 heads' Q/K/V projections, causal attention, and a
# partial output projection (its slice of Wo). Host sums the 4 partials per
# batch and adds the bias.
#
# Device layouts (per core):
#   qT/kT/vT : [E, T] bf16   (host pre-transposes inputs; E on partitions)
#   QT/KT    : [128, T]      head-pair-stacked q^T/k^T (rows 0-63 head 2p, 64-127 head 2p+1)
#   V        : [128, 16*65]  v tiles per head with an appended ones column
#   S^T      : [Tk=128, Tq<=512] per block -> exp -> P^T in SBUF.
#   PV       : operand-swapped matmul: lhsT = P^T chunk [128k, 128q] (stationary),
#              rhs = [V|ones] [128k, 65] -> out [128 q, 65] PSUM accumulated over
#              k-blocks; col 64 is the softmax denominator. Normalize on DVE with
#              a per-partition reciprocal, PE-transpose back to [c, q] for Wo.
# Softmax skips max-subtraction: energies are pre-scaled by 1/sqrt(DH) (folded
# into Wq on host) and are O(+-10), far from fp32 exp overflow.

import os
from contextlib import ExitStack

import ml_dtypes
import numpy as np

import concourse.bass as bass
import concourse.mybir as mybir
import concourse.tile as tile
from concourse import bacc
from concourse.bass_utils import run_bass_kernel_spmd

N, T, E, H, DH = 2, 2048, 1024, 16, 64
P = 128
KC = E // P          # 8 contraction chunks for projections
TB = T // P          # 16 token blocks of 128
T4 = T // 512        # 4 token blocks of 512
HPC = 4              # heads per core
NCORES = 8
BF = mybir.dt.bfloat16
F32 = mybir.dt.float32
EXP = mybir.ActivationFunctionType.Exp

_PROG_CACHE: dict = {}


def _emit(ctx: ExitStack, tc: "tile.TileContext", io: dict, variant: str):
    nc = tc.nc
    const = ctx.enter_context(tc.tile_pool(name="const", bufs=1))
    xin = ctx.enter_context(tc.tile_pool(name="xin", bufs=1))
    proj = ctx.enter_context(tc.tile_pool(name="proj", bufs=1))
    work = ctx.enter_context(tc.tile_pool(name="work", bufs=4))
    small = ctx.enter_context(tc.tile_pool(name="small", bufs=2))
    # PSUM budget (8 banks):
    #   ps_s   : 3 bufs x [128,512] f32 = 3 banks (S blocks, one per head)
    #   ps_acc : 3 tiles (455/455/512 f32) = 3 banks -- 16 PV slots of 65 cols
    #            + 2 transpose slots in tile 2's tail
    #   ps_aux : 1 buf x [128,512] f32 = 1 bank (projections / Wo)
    ps_s = ctx.enter_context(tc.tile_pool(name="ps_s", bufs=3, space="PSUM"))
    ps_acc = ctx.enter_context(tc.tile_pool(name="ps_acc", bufs=1, space="PSUM"))
    ps_aux = ctx.enter_context(tc.tile_pool(name="ps_aux", bufs=2, space="PSUM"))

    # ---- load weights & constants (emission order ~ priority order) ----
    def load_w(srcname, p):
        w_t = const.tile([P, KC * P], BF, tag=f"{srcname}{p}", name=f"{srcname}{p}")
        nc.sync.dma_start(w_t, io[srcname][p])
        return w_t

    wq_sb = [load_w("wq", p) for p in range(2)]

    def alloc_xt(key):
        return xin.tile([P, KC * T], BF, tag=key, name=key)

    def xt_ap(t, kc, lo, hi):  # [128, hi-lo] chunk kc token-slice
        return t[:, kc * T + lo: kc * T + hi]

    def load_xt_t4(t, key, t4, eng=None, halves=False):
        eng = eng or nc.sync
        view_d = t.rearrange("p (c t) -> p c t", c=KC)
        view_s = io[key].rearrange("(c p) t -> p c t", p=P)
        if halves:
            nq = 4 if halves == "quarters" else 2
            w = KC // nq
            for h in range(nq):
                dst = view_d[:, w * h:w * h + w, t4 * 512:(t4 + 1) * 512]
                src = view_s[:, w * h:w * h + w, t4 * 512:(t4 + 1) * 512]
                eng.dma_start(dst, src)
        else:
            eng.dma_start(view_d[:, :, t4 * 512:(t4 + 1) * 512],
                          view_s[:, :, t4 * 512:(t4 + 1) * 512])

    qT_sb = alloc_xt("qT")
    kT_sb = alloc_xt("kT")
    vT_sb = alloc_xt("vT")
    wv_sb = const.tile([P, KC * 256], BF, tag="wv", name="wv")
    # Startup: spread first-stripe loads over two DGE queues so the first
    # projection groups wait on ~1 MB, not the whole input set.  sc/idn are
    # tiny and needed by the first attention block's mask/transpose.
    load_xt_t4(qT_sb, "qT", 0, eng=nc.scalar, halves="quarters")
    if variant == "causal":
        sc_sb = const.tile([P, P], BF, tag="sc", name="sc")
        nc.sync.dma_start(sc_sb, io["sc"])
    wk_sb = [load_w("wk", p) for p in range(2)]
    idn_sb = const.tile([P, P], BF, tag="idn", name="idn")
    nc.sync.dma_start(idn_sb, io["idn"])
    load_xt_t4(kT_sb, "kT", 0, halves=True)
    nc.sync.dma_start(wv_sb, io["wv"])
    load_xt_t4(vT_sb, "vT", 0, halves=True)
    woT_sb = []
    for cc in range(2):
        w_t = const.tile([P, E], BF, tag=f"woT{cc}", name=f"woT{cc}")
        nc.sync.dma_start(w_t, io["woT"][cc * P:(cc + 1) * P, :])
        woT_sb.append(w_t)
    for t4 in range(1, T4):
        load_xt_t4(qT_sb, "qT", t4)
        load_xt_t4(kT_sb, "kT", t4)
        load_xt_t4(vT_sb, "vT", t4)

    # ---- persistent SBUF tensors ----
    QT = [proj.tile([P, T], BF, tag=f"QT{p}", name=f"QT{p}") for p in range(2)]
    KT = [proj.tile([P, T], BF, tag=f"KT{p}", name=f"KT{p}") for p in range(2)]
    V4 = proj.tile([P, TB * HPC * 65], BF, tag="V4", name="V4")

    def V_ap(h, jk):  # [128, 65] rhs for the PV matmul of head h, k-block jk
        return V4[:, jk * (HPC * 65) + h * 65: jk * (HPC * 65) + h * 65 + 65]

    CT = [proj.tile([P, T], BF, tag=f"CT{p}", name=f"CT{p}") for p in range(2)]

    v3 = V4.rearrange("p (b h c) -> p b h c", b=TB, h=HPC)
    nc.vector.memset(v3[:, :, :, 64:65], 1.0)

    # PV accumulators: 16 slots of [128, 65] f32 packed into 3 bank-sized
    # tiles (7 + 7 + 2 slots); slot = p*8 + c*2 + hh.  PSUM allows only one
    # OPEN accumulation group per 2 KB bank, so per stripe each tile hosts a
    # single group: start on the tile's first emitted PV matmul, stop on its
    # last (slots inside accumulate independently via per-element has_written).
    po_t = [ps_acc.tile([P, 455], F32, tag="po0", name="po0"),
            ps_acc.tile([P, 455], F32, tag="po1", name="po1"),
            ps_acc.tile([P, 130], F32, tag="po2", name="po2")]

    def po_slot(p, c, hh):
        s = p * 8 + c * 2 + hh
        return (0, s) if s < 7 else (1, s - 7) if s < 14 else (2, s - 14)

    def po_ap(p, c, hh, width=65):
        t, i = po_slot(p, c, hh)
        return po_t[t][:, i * 65: i * 65 + width]

    def tr_ap():
        return ps_aux.tile([P, 512], F32, tag="aux", name="tr")[:, 0:64].bitcast(BF)

    # ---------------- emission units ----------------
    def emit_qk_proj(t4, which, p):
        # one [128, 512] projection group: 8 accumulating matmuls
        dst, w_sb, x_sb = ((QT[p], wq_sb[p], qT_sb), (KT[p], wk_sb[p], kT_sb))[which]
        ps = ps_aux.tile([P, 512], F32, tag="aux", name="ps_proj")
        for kc in range(KC):
            nc.tensor.matmul(
                ps,
                w_sb[:, kc * P:(kc + 1) * P],
                xt_ap(x_sb, kc, t4 * 512, (t4 + 1) * 512),
                start=(kc == 0),
                stop=(kc == KC - 1),
            )
        nc.vector.tensor_copy(dst[:, t4 * 512:(t4 + 1) * 512], ps)

    def emit_v_proj(tb):
        pv = ps_aux.tile([P, 512], F32, tag="aux", name="ps_v")
        for kc in range(KC):
            nc.tensor.matmul(
                pv[:, 0:256],
                xt_ap(vT_sb, kc, tb * P, (tb + 1) * P),
                wv_sb[:, kc * 256:(kc + 1) * 256],
                start=(kc == 0),
                stop=(kc == KC - 1),
            )
        nc.vector.tensor_copy(v3[:, tb, :, 0:64],
                              pv[:, 0:256].rearrange("p (h c) -> p h c", h=HPC))

    yt_live = {}

    drain_mode = [False]

    def emit_wo_es(tb, es):
        # half of the output projection for one finished 128-token block
        if es == 0:
            yt_live[tb] = work.tile([P, 1024], BF, tag="yt", name="yt", bufs=4)
        yt = yt_live[tb]
        if drain_mode[0]:
            py = ps_s.tile([P, 512], F32, tag="s", name="py")
        else:
            py = ps_aux.tile([P, 512], F32, tag="aux", name="py")
        for cc in range(2):
            nc.tensor.matmul(
                py,
                CT[cc][:, tb * P:(tb + 1) * P],
                woT_sb[cc][:, es * 512:(es + 1) * 512],
                start=(cc == 0),
                stop=(cc == 1),
            )
        if drain_mode[0] and es == 0:
            nc.scalar.copy(yt[:, 0:512], py)
        else:
            nc.vector.tensor_copy(yt[:, es * 512:(es + 1) * 512], py)
        nc.scalar.dma_start(io["y"][tb * P:(tb + 1) * P, es * 512:(es + 1) * 512],
                            yt[:, es * 512:(es + 1) * 512])
        if es == 1:
            del yt_live[tb]

    # ---- filler management: units of PE work to interleave into attention ----
    filler: list = []          # list of (est_pe_ns, ready_ns, closure)
    est_pe = [0.0]             # cumulative emitted PE ns (attention + filler)
    est_act = [0.0]            # cumulative emitted ACT ns
    blocks_left = [1]

    def force_units(key):
        # hard-emit all filler units tagged `key` (correctness: their writes
        # must precede the attention reads that need them)
        for ent in [e for e in filler if len(e) > 4 and e[4] == key]:
            filler.remove(ent)
            ent[2]()
            est_pe[0] += ent[0]

    def pump(margin):
        # Keep emitted PE work >= emitted ACT work + margin so the in-order
        # PE stream never starves while exp runs, and drain the backlog
        # early enough that the post-attention tail is empty.  Units whose
        # input DMA has likely not landed yet (ready_ns) are deferred.
        now = max(est_pe[0], est_act[0]) + 10500.0
        pops = 0
        while filler and pops < 4 and (est_pe[0] < est_act[0] + margin
                                       or len(filler) > blocks_left[0]):
            idx = next((i for i, e in enumerate(filler)
                        if e[1] <= now and (len(e) < 4 or e[3] is None or e[3]())), None)
            if idx is None:
                break
            ent = filler.pop(idx)
            ent[2]()
            est_pe[0] += ent[0]
            pops += 1

    # Deferred transposes: emitted a couple of chunk-completions later (or
    # pulled in by the Wo unit that needs them) so the in-order PE stream
    # doesn't wait on the DVE normalize round-trip.
    pending_tr: dict = {}
    tr_q: list = []
    tr_done_at: dict = {}
    fc_ctr = [0]

    def emit_transpose(key):
        fn = pending_tr.pop(key, None)
        if fn is None:
            return
        tr_q.remove(key)
        fn()
        tr_done_at[key] = fc_ctr[0]
        est_pe[0] += 53.0

    rem_at = {1: 64, 2: 48, 3: 24}

    def finish_chunk(p, tb):
        fc_ctr[0] += 1
        # flush old transposes first so the x2 ring (bufs=4) can never
        # cycle through an unemitted PE consumer
        while len(tr_q) > 4:
            emit_transpose(tr_q[0])
        c = tb % 4
        rec = small.tile([P, 2], F32, tag="rec", name="rec", bufs=6)
        for hh in range(2):
            nc.vector.reciprocal(rec[:, hh:hh + 1], po_ap(p, c, hh)[:, 64:65])
        x2 = small.tile([P, P], F32, tag="x2", name="x2", bufs=6)
        for hh in range(2):
            nc.vector.tensor_scalar_mul(
                x2[:, hh * 64:(hh + 1) * 64],
                po_ap(p, c, hh, width=64),
                rec[:, hh:hh + 1],
            )

        def do_tr(p=p, tb=tb, x2=x2):
            tr = tr_ap()
            nc.tensor.transpose(tr, x2, idn_sb)
            nc.vector.tensor_copy(CT[p][:, tb * P:(tb + 1) * P], tr)

        key = (p, tb)
        pending_tr[key] = do_tr
        tr_q.append(key)
        if p == 1:
            def wo0(tb=tb):
                emit_transpose((0, tb))
                emit_transpose((1, tb))
                emit_wo_es(tb, 0)

            gate = rem_at.get(tb // 4 + 1, 10 ** 9)

            def wo0_ready(tb=tb, gate=gate):
                if drain_mode[0]:
                    return True

                for p_ in range(2):
                    if (p_, tb) in pending_tr:
                        return False
                    if fc_ctr[0] < tr_done_at.get((p_, tb), 0) + 1:
                        return False
                return True
            filler.append((533.0, 18000.0, wo0, wo0_ready))
            filler.append((427.0, 18000.0, lambda tb=tb: emit_wo_es(tb, 1),
                           lambda tb=tb: tb in yt_live))

    PRE_JK = 9  # stripe-3 k-blocks whose S/exp precompute as filler
    pt3 = {}
    if variant == "causal":
        for jk in range(PRE_JK):
            for p in range(2):
                pt3[(p, jk)] = proj.tile([P, 1024], BF, tag=f"pt3_{p}_{jk}",
                                         name=f"pt3_{p}_{jk}")

    def emit_pre_s(p, jk):
        # stripe-3 S + exp for a full block, into the persistent pt3 tile
        iq = T4 - 1
        pt = pt3[(p, jk)]
        for hh in range(2):
            ps2 = ps_s.tile([P, 512], F32, tag="s", name="ps_s")
            nc.tensor.matmul(
                ps2,
                KT[p][hh * 64:(hh + 1) * 64, jk * P:(jk + 1) * P],
                QT[p][hh * 64:(hh + 1) * 64, iq * 512:(iq + 1) * 512],
                start=True,
                stop=True,
            )
            est_pe[0] += 512 * 0.42
            nc.scalar.activation(pt[:, hh * 512:(hh + 1) * 512], ps2, EXP)
            est_act[0] += 512 * 0.833 + 185

    def emit_attention(iq, margin):
        njk = 4 * iq + 4 if variant == "causal" else TB
        lag = 2
        pend = {0: [], 1: []}  # per-p [(pt, co, jk)] awaiting PV

        # dry-run the PV emission order to place one start/stop per po tile
        seq = []

        def chunk_range(jk):
            r = jk - 4 * iq
            cstart = max(r, 0) if variant == "causal" else 0
            return range(3, cstart - 1, -1)

        for ljk in range(njk):
            for p_ in range(2):
                if ljk >= lag:
                    for c in chunk_range(ljk - lag):
                        for hh in range(2):
                            seq.append((p_, ljk - lag, c, hh))
        for p_ in range(2):
            for jk_ in range(max(njk - lag, 0), njk):
                for c in chunk_range(jk_):
                    for hh in range(2):
                        seq.append((p_, jk_, c, hh))
        first_mm, last_mm = {}, {}
        for ent in seq:
            t, _ = po_slot(ent[0], ent[2], ent[3])
            first_mm.setdefault(t, ent)
            last_mm[t] = ent
        first_set = set(first_mm.values())
        last_set = set(last_mm.values())

        def flush(item, p, iq=iq, njk=njk):
            pt, co, jk = item
            for c in chunk_range(jk):
                jq = 4 * iq + c
                complete = (jk == jq) if variant == "causal" else (jk == njk - 1)
                for hh in range(2):
                    nc.tensor.matmul(
                        po_ap(p, c, hh),
                        pt[:, hh * 512 + c * P: hh * 512 + (c + 1) * P],
                        V_ap(2 * p + hh, jk),
                        start=((p, jk, c, hh) in first_set),
                        stop=((p, jk, c, hh) in last_set),
                        skip_group_check=True,
                    )
                est_pe[0] += 2 * 65 * 0.42
                if complete:
                    finish_chunk(p, jq)

        force_units(("q", iq))
        for jk in range(njk):
            if jk == max(4 * iq - 2, 0):
                # K/V of this stripe must be emitted before the diagonal
                # blocks (S reads KT stripe iq at jk=4iq; PV reads V there
                # too, two blocks later)
                force_units(("k", iq))
                force_units(("v", iq))
            for p in range(2):
                r = jk - 4 * iq
                co = P * r if (variant == "causal" and r >= 0) else 0
                if variant == "causal" and iq == T4 - 1 and jk < PRE_JK:
                    pend[p].append((pt3[(p, jk)], 0, jk))
                    if len(pend[p]) > lag:
                        flush(pend[p].pop(0), p)
                    blocks_left[0] -= 1
                    pump(margin)
                    continue
                pt = work.tile([P, 1024], BF, tag="pt", name="pt", bufs=7)
                for hh in range(2):
                    ps2 = ps_s.tile([P, 512], F32, tag="s", name="ps_s")
                    nc.tensor.matmul(
                        ps2[:, co:512],
                        KT[p][hh * 64:(hh + 1) * 64, jk * P:(jk + 1) * P],
                        QT[p][hh * 64:(hh + 1) * 64, iq * 512 + co:(iq + 1) * 512],
                        start=True,
                        stop=True,
                    )
                    est_pe[0] += (512 - co) * 0.42
                    nc.scalar.activation(pt[:, hh * 512 + co:(hh + 1) * 512],
                                         ps2[:, co:512], EXP)
                    est_act[0] += (512 - co) * 0.833 + 185
                pump(margin)
                if variant == "causal" and r >= 0:
                    for hh in range(2):
                        sl = pt[:, hh * 512 + co: hh * 512 + co + P]
                        nc.gpsimd.tensor_mul(sl, sl, sc_sb)
                elif variant == "mask":
                    mk = work.tile([P, 512], BF, tag="mk", name="mk", bufs=6)
                    nc.sync.dma_start(mk, io["mT"][jk * P:(jk + 1) * P, iq * 512:(iq + 1) * 512])
                    for hh in range(2):
                        sl = pt[:, hh * 512:(hh + 1) * 512]
                        nc.gpsimd.tensor_mul(sl, sl, mk)
                pend[p].append((pt, co, jk))
                if len(pend[p]) > lag:
                    flush(pend[p].pop(0), p)
                blocks_left[0] -= 1
                pump(margin)
        for p in range(2):
            while pend[p]:
                flush(pend[p].pop(0), p)
                pump(margin)

    # ---------------- schedule ----------------
    if variant == "causal":
        # stripe-0 projections emitted directly (nothing to overlap with yet)
        for p in range(2):
            emit_qk_proj(0, 0, p)
        for p in range(2):
            emit_qk_proj(0, 1, p)
        for tb in range(4):
            emit_v_proj(tb)
        # later-stripe projections become filler for the attention phase.
        # ready_ns ~ cumulative serial DMA transfer time when that stripe's
        # tensor has landed (weights+stripe0 ~ 14.3us, then 3.16us per load).
        for t4 in range(1, T4):
            rq = 16200.0 + (t4 - 1) * 8736.0
            rk = rq + 2912.0
            rv = rk + 2912.0
            gate = rem_at[t4]
            for p in range(2):
                filler.append((1707.0, rq, lambda t4=t4, p=p: emit_qk_proj(t4, 0, p),
                               None, ("q", t4)))
            for p in range(2):
                filler.append((1707.0, rk, lambda t4=t4, p=p: emit_qk_proj(t4, 1, p),
                               lambda g=gate: blocks_left[0] <= g, ("k", t4)))
            for tb in range(4 * t4, 4 * t4 + 4):
                filler.append((853.0, rv, lambda tb=tb: emit_v_proj(tb),
                               lambda g=gate: blocks_left[0] <= g, ("v", t4)))
        blocks_left[0] = sum(2 * (4 * iq + 4) for iq in range(T4))
        q3_done = [0]
        _orig_qk = emit_qk_proj

        def emit_qk_proj_w(t4, which, p):
            _orig_qk(t4, which, p)
            if t4 == T4 - 1 and which == 0:
                q3_done[0] += 1

        emit_qk_proj = emit_qk_proj_w  # noqa: F811 (rebind for filler closures)
        for jk in range(PRE_JK):
            for p in range(2):
                filler.append((427.0, 30000.0,
                               lambda p=p, jk=jk: emit_pre_s(p, jk),
                               lambda: q3_done[0] == 2, ("pre3",)))
        for iq in range(T4):
            if iq == T4 - 1:
                force_units(("pre3",))
            emit_attention(iq, (2000.0, 2000.0, 1600.0, 3600.0)[iq])
        drain_mode[0] = True
        while filler:
            filler.pop(0)[2]()
    else:
        for t4 in range(T4):
            for p in range(2):
                emit_qk_proj(t4, 0, p)
                emit_qk_proj(t4, 1, p)
            for tb in range(4 * t4, 4 * t4 + 4):
                emit_v_proj(tb)
        blocks_left[0] = 2 * TB * T4
        for iq in range(T4):
            emit_attention(iq, 2500.0)
        drain_mode[0] = True
        while filler:
            filler.pop(0)[2]()


def build_program(variant: str):
    if variant in _PROG_CACHE:
        return _PROG_CACHE[variant]
    nc = bacc.Bacc("TRN2", target_bir_lowering=False, debug=False, num_devices=NCORES)
    io = {
        "qT": nc.dram_tensor("qT", [E, T], BF, kind="ExternalInput").ap(),
        "kT": nc.dram_tensor("kT", [E, T], BF, kind="ExternalInput").ap(),
        "vT": nc.dram_tensor("vT", [E, T], BF, kind="ExternalInput").ap(),
        "wq": nc.dram_tensor("wq", [2, P, KC * P], BF, kind="ExternalInput").ap(),
        "wk": nc.dram_tensor("wk", [2, P, KC * P], BF, kind="ExternalInput").ap(),
        "wv": nc.dram_tensor("wv", [P, KC * 256], BF, kind="ExternalInput").ap(),
        "woT": nc.dram_tensor("woT", [256, E], BF, kind="ExternalInput").ap(),
        "idn": nc.dram_tensor("idn", [P, P], BF, kind="ExternalInput").ap(),
        "y": nc.dram_tensor("y", [T, E], BF, kind="ExternalOutput").ap(),
    }
    if variant == "causal":
        io["sc"] = nc.dram_tensor("sc", [P, P], BF, kind="ExternalInput").ap()
    elif variant == "mask":
        io["mT"] = nc.dram_tensor("mT", [T, T], BF, kind="ExternalInput").ap()
    with tile.TileContext(nc) as tc:
        with ExitStack() as ctx:
            _emit(ctx, tc, io, variant)
    nc.compile()
    _PROG_CACHE[variant] = nc
    return nc


def make_in_maps(query, key, value, mask, Wq, Wk, Wv, Wo, variant):
    """Build the 8 per-core input maps (host-side sharding + layout prep)."""
    bf = ml_dtypes.bfloat16
    scale = np.float32(1.0 / np.sqrt(DH))
    Wq = np.asarray(Wq, np.float32) * scale
    Wk = np.asarray(Wk, np.float32)
    Wv = np.asarray(Wv, np.float32)
    Wo = np.asarray(Wo, np.float32)

    xT = {}
    for name, x in (("qT", query), ("kT", key), ("vT", value)):
        xT[name] = [np.ascontiguousarray(np.asarray(x[n], np.float32).T).astype(bf) for n in range(N)]
    mT = None
    if variant == "mask":
        mT = [np.ascontiguousarray((np.asarray(mask[n, 0]) != 0).T).astype(bf) for n in range(N)]
    sc = None
    if variant == "causal":
        sc = np.zeros((P, P), np.float32)
        for prt in range(P):
            sc[prt, prt:] = 1.0
        sc = sc.astype(bf)
    idn = np.eye(P, dtype=np.float32).astype(bf)

    per_c4 = []
    for c4 in range(4):
        heads = [4 * c4 + i for i in range(4)]
        def swz(w):  # [E, cols] -> [128, KC*cols] partition-swizzled
            cols = w.shape[1]
            return np.ascontiguousarray(
                w.reshape(KC, P, cols).transpose(1, 0, 2).reshape(P, KC * cols))

        wq = np.stack([
            swz(np.concatenate([Wq[heads[2 * p]], Wq[heads[2 * p + 1]]], axis=1)) for p in range(2)
        ]).astype(bf)
        wk = np.stack([
            swz(np.concatenate([Wk[heads[2 * p]], Wk[heads[2 * p + 1]]], axis=1)) for p in range(2)
        ]).astype(bf)
        wv = swz(np.concatenate([Wv[h] for h in heads], axis=1)).astype(bf)
        woT = np.ascontiguousarray(Wo[:, c4 * 256:(c4 + 1) * 256].T).astype(bf)
        per_c4.append((wq, wk, wv, woT))

    in_maps = []
    for core in range(NCORES):
        n, c4 = divmod(core, 4)
        wq, wk, wv, woT = per_c4[c4]
        im = {
            "qT": xT["qT"][n], "kT": xT["kT"][n], "vT": xT["vT"][n],
            "wq": wq, "wk": wk, "wv": wv, "woT": woT, "idn": idn,
        }
        if variant == "causal":
            im["sc"] = sc
        elif variant == "mask":
            im["mT"] = mT[n]
        in_maps.append(im)
    return in_maps


def detect_variant(mask):
    m = np.asarray(mask) != 0
    if m.all():
        return "full"
    tril = np.tril(np.ones((T, T), dtype=bool))
    if all(np.array_equal(m[n, 0], tril) for n in range(N)):
        return "causal"
    return "mask"


def kernel_run(query, key, value, mask, Wq, Wk, Wv, Wo, bo, trace=False):
    variant = detect_variant(mask)
    nc = build_program(variant)
    in_maps = make_in_maps(query, key, value, mask, Wq, Wk, Wv, Wo, variant)
    try:
        res = run_bass_kernel_spmd(nc, in_maps, core_ids=list(range(NCORES)), trace=trace)
    except ModuleNotFoundError:
        res = run_bass_kernel_spmd(nc, in_maps, core_ids=list(range(NCORES)))
    bo = np.asarray(bo, np.float32)
    out = np.empty((N, T, E), np.float32)
    for n in range(N):
        acc = np.zeros((T, E), np.float32)
        for c4 in range(4):
            acc += np.asarray(res.results[4 * n + c4]["y"], np.float32)
        out[n] = acc + bo
    return out, res


def kernel(query, key, value, mask, Wq, Wk, Wv, Wo, bo):
    out, _ = kernel_run(query, key, value, mask, Wq, Wk, Wv, Wo, bo)
    return out
